# revision 44
# baseline (speedup 1.0000x reference)
"""DCRNN kernel for 8 Trainium2 NeuronCores (Bass/Tile).

Graph/data-parallel sharding (per hint): nodes permuted so core c owns
batch-lanes [c*125,(c+1)*125) of every graph; edges partitioned by dst shard.
conv1 aggregation uses a host-prepared dst-aligned layered table (layer k =
x4 of the k-th edge of each dst) so on-device it is a stack of wide DVE adds
— no gather, no one-hot.  The normalized 4-wide agg is AllGathered (tiny);
every core recomputes full h1 and writes a p-major bf16 h1 table (12.5KB DMA
descriptors).  conv2 buckets edges by (dst-group, src core-pair block) with
cross-core-uniform chunk counts; one dma_gather per (supergroup, block) run
(~5k indices amortizes SWDGE fixed cost) + one-hot matmul scatter in PSUM.
LSTM runs transposed-gates (batch 125/core) with bf16 weights, all 4 gates
in one PSUM tile with DMA-preloaded biases (3 activations/step); global mean
pool via free-dim reduce + AllReduce.
"""
import numpy as np
import ml_dtypes

BF16 = ml_dtypes.bfloat16

N = 100000
NPG = 1000
B_GRAPHS = 100
H = 128
CIN = 3
OUT = 2
NCORES = 8
SH = 12500          # nodes per core
NG = 98             # dst groups of 128 per core (last group = 84 nodes)
SHPAD = NG * 128    # 12544
NB = 4              # src blocks (core pairs) for conv2 gather
BLK2 = 2 * SHPAD    # 25088 rows per block in h1tab2 (int16-indexable)
GS = 8              # dst groups per super-group (conv2)
GCALL = 896         # idxs per dma_gather call
QW = 392            # quad-packed layer width (128 partitions * 392 = 4*12544)
T = 100
BL = 125            # batch lanes per core

_BUILT = {}


# --------------------------------------------------------------------------
# host preprocessing
# --------------------------------------------------------------------------
def _perm():
    n = np.arange(N)
    c = (n % NPG) // BL
    return c * SH + (n // NPG) * BL + (n % NPG) % BL


def _host_prep(inputs):
    x = np.asarray(inputs["x"], np.float32)
    ei = np.asarray(inputs["edge_index"])
    src, dst = ei[0].astype(np.int64), ei[1].astype(np.int64)
    p = _perm()
    srcp = p[src]
    dstp = p[dst]

    deg = np.bincount(dstp, minlength=N).astype(np.float32)
    recip = 1.0 / np.maximum(deg, 1.0)
    L = int(deg.max())                      # layers for conv1 dst-aligned agg

    # x in perm order, padded with a ones column (bias via W row)
    inv = np.empty(N, np.int64)
    inv[p] = np.arange(N)
    x4 = np.zeros((N, 4), np.float32)
    x4[:, :CIN] = x[inv]
    x4[:, CIN] = 1.0
    x4T = np.ascontiguousarray(x4.T)        # [4, N]
    x4Tp = np.zeros((4, NCORES * SHPAD), np.float32)  # SHPAD-padded per rank
    for c in range(NCORES):
        x4Tp[:, c * SHPAD:c * SHPAD + SH] = x4T[:, c * SH:(c + 1) * SH]

    owner = dstp // SH
    Ldst = dstp - owner * SH                # within-shard dst position

    # conv1: dst-aligned layered tables.  rank_within_dst via stable sort.
    order_d = np.argsort(dstp, kind="stable")
    d_sorted = dstp[order_d]
    s_sorted = srcp[order_d]
    run_start = np.zeros(N, np.int64)
    run_start[1:] = np.cumsum(np.bincount(dstp, minlength=N))[:-1]
    rank_d = np.arange(len(d_sorted)) - run_start[d_sorted]

    # h1tab2 p-major row ids: node (c, s) -> c*SHPAD + (s%128)*98 + s//128
    row2 = (srcp // SH) * SHPAD + (srcp % SH % 128) * 98 + (srcp % SH) // 128

    # conv2 bucketing (per core): (dst group g, src block b = row2//BLK2)
    K = np.zeros((NG, NB), np.int64)
    per_core = []
    for c in range(NCORES):
        m = owner == c
        Lc = Ldst[m]
        g = Lc // 128
        slot = (Lc % 128).astype(np.float32)
        r2 = row2[m]
        b = r2 // BLK2
        s16 = (r2 % BLK2).astype(np.int16)
        key = (g * NB + b).astype(np.int64)
        order = np.argsort(key, kind="stable")
        cnt = np.bincount(key, minlength=NG * NB)
        per_core.append((s16[order], slot[order], key[order], cnt))
        K = np.maximum(K, ((cnt + 127) // 128).reshape(NG, NB))
    K = np.maximum(K, 1)

    # chunk layout: for sup: for b: for g in sup: K[g,b] chunks
    sups = [range(i, min(i + GS, NG)) for i in range(0, NG, GS)]
    chunk_base = np.zeros((NG, NB), np.int64)
    gmeta = []
    nch = 0
    for sup in sups:
        bruns = []
        for b in range(NB):
            run_base = nch
            for g in sup:
                chunk_base[g, b] = nch
                nch += K[g, b]
            bruns.append((b, run_base, nch - run_base))
        gmeta.append(bruns)
    NCH = nch
    NSL = NCH * 128

    percore = []
    base_of_key = chunk_base.reshape(-1) * 128
    # quad-pack map for conv1 layers: position s -> part (s//QW)*4+f, col s%QW
    pos = np.arange(SH)
    qa, qc = pos // QW, pos % QW
    for c in range(NCORES):
        s_sorted2, slot_sorted, key_sorted, cnt = per_core[c]
        run_st = np.concatenate([[0], np.cumsum(cnt)[:-1]])
        rank_within = np.arange(len(s_sorted2)) - run_st[key_sorted]
        posn = base_of_key[key_sorted] + rank_within
        idx_flat = np.zeros(NSL, np.int16)
        dm_flat = np.full(NSL, -1.0, np.float32)
        idx_flat[posn] = s_sorted2
        dm_flat[posn] = slot_sorted
        w = idx_flat.reshape(NSL // 16, 16).T
        r = np.ones(SHPAD, np.float32)
        r[:SH] = recip[c * SH:(c + 1) * SH]

        # conv1 layered table [128, L*QW] bf16 + recq [128, QW] f32
        mm = (d_sorted >= c * SH) & (d_sorted < (c + 1) * SH)
        ls = d_sorted[mm] - c * SH          # within-shard dst pos, sorted
        lr = rank_d[mm]                     # layer index per edge
        lsrc = s_sorted[mm]                 # src node (perm id)
        xlay = np.zeros((128, L * QW), np.float32)
        partk = qa[ls] * 4                  # base partition of quad
        colk = lr * QW + qc[ls]
        for f in range(4):
            xlay[partk + f, colk] = x4[lsrc, f]
        recq = np.zeros((128, QW), np.float32)
        for f in range(4):
            recq[qa * 4 + f, qc] = recip[c * SH:(c + 1) * SH]

        xl = np.zeros((4, SHPAD), np.float32)
        xl[:, :SH] = x4T[:, c * SH:(c + 1) * SH]
        percore.append({
            "idx16": np.ascontiguousarray(np.tile(w, (8, 1)).astype(np.int16)),
            "dmv": np.ascontiguousarray(dm_flat.reshape(NCH, 128).T
                                        .astype(BF16)),
            "recrow": np.ascontiguousarray(r.reshape(NG, 128).T
                                           .astype(np.float32)),
            "xlay": np.ascontiguousarray(xlay.astype(BF16)),
            "recq": np.ascontiguousarray(recq),
            "x4tloc": xl.astype(BF16),
        })

    Wcomb = np.zeros((8, H), np.float32)
    Wcomb[0:3] = np.asarray(inputs["W_self0"], np.float32)
    Wcomb[3] = np.asarray(inputs["b0"], np.float32)
    Wcomb[4:7] = np.asarray(inputs["W_nbr0"], np.float32)

    shared = {
        "x4T": x4Tp.astype(BF16),
        "wcomb": Wcomb.astype(BF16),
        "ws1": np.asarray(inputs["W_self1"], np.float32).astype(BF16),
        "wn1": np.asarray(inputs["W_nbr1"], np.float32).astype(BF16),
        "b1c": np.ascontiguousarray(
            np.asarray(inputs["b1"], np.float32).reshape(H, 1)),
        "wo": (np.asarray(inputs["W_out"], np.float32) / NPG)
            .astype(np.float32),
        "bo": np.ascontiguousarray(
            np.asarray(inputs["b_out"], np.float32).reshape(OUT, 1)),
    }
    for l in range(2):
        wi = np.asarray(inputs[f"Wih{l}"], np.float32)
        wh = np.asarray(inputs[f"Whh{l}"], np.float32)
        bs = (np.asarray(inputs[f"bih{l}"], np.float32)
              + np.asarray(inputs[f"bhh{l}"], np.float32))
        # gate order (g, i, f, o): tanh(g) starts while i/f/o matmuls
        # still run; one sigmoid covers cols [125:500]
        GQ = (2, 0, 1, 3)
        shared[f"wi{l}"] = np.ascontiguousarray(np.concatenate(
            [wi[q * H:(q + 1) * H].T for q in GQ], axis=1)).astype(BF16)
        shared[f"wh{l}"] = np.ascontiguousarray(np.concatenate(
            [wh[q * H:(q + 1) * H].T for q in GQ], axis=1)).astype(BF16)
        shared[f"bs4_{l}"] = np.ascontiguousarray(
            bs.reshape(4, H)[list(GQ)].astype(BF16))
    sel4 = np.zeros((4, 4 * BL), np.float32)
    for q in range(4):
        sel4[q, q * BL:(q + 1) * BL] = 1.0
    shared["sel4"] = sel4.astype(BF16)

    meta = (L,) + tuple(K.reshape(-1).tolist())
    return shared, percore, meta, K, gmeta, NCH, chunk_base, L


# --------------------------------------------------------------------------
# device program
# --------------------------------------------------------------------------
def _build_nc(K, gmeta, NCH, chunk_base, L):
    import concourse.bacc as bacc
    import concourse.mybir as mybir
    from concourse.tile import TileContext
    from concourse.masks import make_identity

    f32 = mybir.dt.float32
    bf = mybir.dt.bfloat16
    i16 = mybir.dt.int16
    AF = mybir.ActivationFunctionType
    ALU = mybir.AluOpType
    NSL = NCH * 128
    Kf = K.reshape(NG, NB)
    sups = [range(i, min(i + GS, NG)) for i in range(0, NG, GS)]

    nc = bacc.Bacc(None, target_bir_lowering=False)

    d_x4T = nc.dram_tensor("x4T", [4, NCORES * SHPAD], bf,
                           kind="ExternalInput")
    d_wcomb = nc.dram_tensor("wcomb", [8, H], bf, kind="ExternalInput")
    d_ws1 = nc.dram_tensor("ws1", [H, H], bf, kind="ExternalInput")
    d_wn1 = nc.dram_tensor("wn1", [H, H], bf, kind="ExternalInput")
    d_b1c = nc.dram_tensor("b1c", [H, 1], f32, kind="ExternalInput")
    d_wo = nc.dram_tensor("wo", [H, OUT], f32, kind="ExternalInput")
    d_bo = nc.dram_tensor("bo", [OUT, 1], f32, kind="ExternalInput")
    d_wi = [nc.dram_tensor(f"wi{l}", [H, 4 * H], bf, kind="ExternalInput")
            for l in range(2)]
    d_wh = [nc.dram_tensor(f"wh{l}", [H, 4 * H], bf, kind="ExternalInput")
            for l in range(2)]
    d_bs4 = [nc.dram_tensor(f"bs4_{l}", [4, H], bf, kind="ExternalInput")
             for l in range(2)]
    d_sel4 = nc.dram_tensor("sel4", [4, 4 * BL], bf, kind="ExternalInput")
    d_idx = nc.dram_tensor("idx16", [128, NSL // 16], i16, kind="ExternalInput")
    d_dmv = nc.dram_tensor("dmv", [128, NCH], bf, kind="ExternalInput")
    d_recrow = nc.dram_tensor("recrow", [128, NG], f32, kind="ExternalInput")
    d_xlay = nc.dram_tensor("xlay", [128, L * QW], bf, kind="ExternalInput")
    d_recq = nc.dram_tensor("recq", [128, QW], f32, kind="ExternalInput")
    d_x4tloc = nc.dram_tensor("x4tloc", [4, SHPAD], bf, kind="ExternalInput")
    d_out = nc.dram_tensor("out", [B_GRAPHS, OUT], f32, kind="ExternalOutput")

    with TileContext(nc) as tc:
        with (
            tc.tile_pool(name="dram", bufs=1, space="DRAM") as dramp,
            tc.tile_pool(name="persist", bufs=1) as pers,
        ):
            h1tab2 = dramp.tile([NCORES * SHPAD, H], bf)
            cc_in = dramp.tile([4, SHPAD], bf)
            cc_out = dramp.tile([4 * NCORES, SHPAD], bf, addr_space="Shared")
            ccr_in = dramp.tile([H, B_GRAPHS], bf)
            ccr_out = dramp.tile([NCORES * H, B_GRAPHS], bf,
                                 addr_space="Shared")

            h2T = pers.tile([H, SHPAD], bf)
            h1Tl = pers.tile([H, SHPAD], bf)
            w_comb = pers.tile([8, H], bf)
            w_s1 = pers.tile([H, H], bf)
            w_n1 = pers.tile([H, H], bf)
            b1c = pers.tile([H, 1], f32)
            w_i = [pers.tile([H, 4 * H], bf, name=f"w_i{l}") for l in range(2)]
            w_h = [pers.tile([H, 4 * H], bf, name=f"w_h{l}") for l in range(2)]
            b_s4 = [pers.tile([4, H], bf, name=f"b_s4{l}") for l in range(2)]
            sel4 = pers.tile([4, 4 * BL], bf)
            w_o = pers.tile([H, OUT], f32)
            b_o = pers.tile([OUT, 1], f32)
            recrow = pers.tile([128, NG], f32)
            iotab = pers.tile([128, 128], bf)
            identb = pers.tile([128, 128], bf)
            identf = pers.tile([128, 128], f32)
            pooledT = pers.tile([H, B_GRAPHS], f32)

            nc.sync.dma_start(out=w_comb[:], in_=d_wcomb[:])
            nc.sync.dma_start(out=w_s1[:], in_=d_ws1[:])
            nc.sync.dma_start(out=w_n1[:], in_=d_wn1[:])
            nc.sync.dma_start(out=b1c[:], in_=d_b1c[:])
            for l in range(2):
                nc.sync.dma_start(out=w_i[l][:], in_=d_wi[l][:])
                nc.sync.dma_start(out=w_h[l][:], in_=d_wh[l][:])
                nc.sync.dma_start(out=b_s4[l][:], in_=d_bs4[l][:])
            nc.sync.dma_start(out=sel4[:], in_=d_sel4[:])
            nc.sync.dma_start(out=w_o[:], in_=d_wo[:])
            nc.sync.dma_start(out=b_o[:], in_=d_bo[:])
            nc.sync.dma_start(out=recrow[:], in_=d_recrow[:])
            iota_i = pers.tile([128, 128], mybir.dt.int32)
            nc.gpsimd.iota(iota_i[:], pattern=[[1, 128]], base=0,
                           channel_multiplier=0)
            nc.vector.tensor_copy(out=iotab[:], in_=iota_i[:])
            make_identity(nc, identf[:])
            nc.vector.tensor_copy(out=identb[:], in_=identf[:])

            # ---------------- Phase 1: conv1 layered aggregation ----------
            with (
                tc.tile_pool(name="p1l", bufs=2) as lpool,
                tc.tile_pool(name="p1a", bufs=1) as apool,
                tc.tile_pool(name="p1ps", bufs=1, space="PSUM") as pspool1,
            ):
                recq = apool.tile([128, QW], f32)
                nc.sync.dma_start(out=recq[:], in_=d_recq[:])
                acc = pspool1.tile([128, QW], f32, space="PSUM", tag="acc")
                LC = 8                      # layers per DMA chunk
                for k0 in range(0, L, LC):
                    nl = min(LC, L - k0)
                    lt = lpool.tile([128, LC * QW], bf, tag="lay")
                    nc.sync.dma_start(
                        out=lt[:, :nl * QW],
                        in_=d_xlay[:, k0 * QW:(k0 + nl) * QW])
                    for k in range(nl):
                        nc.tensor.matmul(
                            out=acc[:], lhsT=identb[:],
                            rhs=lt[:, k * QW:(k + 1) * QW],
                            start=(k0 + k == 0), stop=(k0 + k == L - 1))
                accb = apool.tile([128, QW], bf)
                nc.vector.tensor_tensor(out=accb[:], in0=acc[:],
                                        in1=recq[:], op=ALU.mult)
                # unpack quad layout [(a f), c] -> agg4T [f, (a c)] via a
                # DRAM bounce (a 2-level partition split is not a valid
                # SBUF access pattern, but is a plain strided DRAM AP)
                qscr = dramp.tile([128, QW], bf)
                nc.sync.dma_start(out=qscr[:], in_=accb[:])
                nc.sync.dma_start(
                    out=cc_in[:].rearrange("f (a c) -> f a c", a=32),
                    in_=qscr[:].rearrange("(a f) c -> f a c", f=4))

            nc.gpsimd.collective_compute(
                "AllGather", mybir.AluOpType.bypass,
                replica_groups=[list(range(NCORES))],
                ins=[cc_in.opt()], outs=[cc_out.opt()],
            )

            # -------- Phase 3: recompute h1 (all ranks), p-major table ----
            with (
                tc.tile_pool(name="p3xa", bufs=2) as xapool,
                tc.tile_pool(name="p3x1", bufs=1) as xapool1,
                tc.tile_pool(name="p3h", bufs=2) as hpool,
                tc.tile_pool(name="p3ps", bufs=3, space="PSUM") as pspool3,
            ):
                # own-shard transposed h1 — needs only local agg4T, so it
                # overlaps the AllGather above
                xal = xapool1.tile([8, SHPAD], bf, tag="xal")
                nc.sync.dma_start(out=xal[0:4, :], in_=d_x4tloc[:])
                nc.sync.dma_start(out=xal[4:8, :], in_=cc_in[:])
                for j0 in range(0, NG, 4):
                    nj = min(4, NG - j0)
                    ps = pspool3.tile([H, 4 * 128], f32, space="PSUM",
                                      tag="psl")
                    for j in range(j0, j0 + nj):
                        nc.tensor.matmul(
                            out=ps[:, (j - j0) * 128:(j - j0 + 1) * 128],
                            lhsT=w_comb[:],
                            rhs=xal[:, j * 128:(j + 1) * 128],
                            start=True, stop=True)
                    nc.scalar.activation(
                        h1Tl[:, j0 * 128:(j0 + nj) * 128],
                        ps[:, :nj * 128], AF.Relu)

                for r in range(NCORES):
                    xa = xapool.tile([8, SHPAD], bf, tag="xa")
                    nc.sync.dma_start(out=xa[0:4, :],
                                      in_=d_x4T[:, r * SHPAD:(r + 1) * SHPAD])
                    nc.sync.dma_start(out=xa[4:8, :],
                                      in_=cc_out[4 * r:4 * r + 4, :])
                    for half in range(2):
                        h0 = half * 49
                        hb = hpool.tile([128, 49 * 128], bf, tag="hrow")
                        for bi, j0 in enumerate(range(h0, h0 + 49, 4)):
                            nj = min(4, h0 + 49 - j0)
                            ps = pspool3.tile([128, 4 * H], f32, space="PSUM",
                                              tag="psr")
                            for j in range(j0, j0 + nj):
                                nc.tensor.matmul(
                                    out=ps[:, (j - j0) * H:(j - j0 + 1) * H],
                                    lhsT=xa[:, j * 128:(j + 1) * 128],
                                    rhs=w_comb[:], start=True, stop=True)
                            dst = hb[:, (j0 - h0) * 128:(j0 - h0 + nj) * 128]
                            if bi % 2 == 0:
                                nc.scalar.activation(dst, ps[:, :nj * H],
                                                     AF.Relu)
                            else:
                                nc.vector.tensor_scalar(
                                    out=dst, in0=ps[:, :nj * H], scalar1=0.0,
                                    scalar2=None, op0=ALU.max)
                        # p-major write: rows p*98+h0+(0..48), 12.5KB/desc
                        nc.sync.dma_start(
                            out=h1tab2[r * SHPAD:(r + 1) * SHPAD, :]
                                .rearrange("(p j) h -> p (j h)", p=128)
                                [:, h0 * 128:(h0 + 49) * 128],
                            in_=hb[:, :])

            # ---------------- Phase 4: conv2 -----------------------------
            with (
                tc.tile_pool(name="p4g", bufs=3) as gpool,
                tc.tile_pool(name="p4oh", bufs=2) as ohpool,
                tc.tile_pool(name="p4m", bufs=3) as mpool,
                tc.tile_pool(name="p4ps", bufs=2, space="PSUM") as pspool4a,
                tc.tile_pool(name="p4ps2", bufs=2, space="PSUM") as pspool4b,
                tc.tile_pool(name="p4t", bufs=3) as tpool,
            ):
                KRMAX = max(kr for bruns in gmeta for _, _, kr in bruns)
                agg2acc = pers.tile([128, NG * 128], bf)
                # b-major: block b's gathers depend only on h1 of cores
                # 2b/2b+1, overlapping conv2 with the tail of phase 3
                for b in range(NB):
                    for bruns, sup in zip(gmeta, sups):
                        _, run_base, kr = bruns[b]
                        n_idx = kr * 128
                        it = mpool.tile([128, (KRMAX * 128) // 16], i16,
                                        tag="idx")
                        nc.sync.dma_start(
                            out=it[:, :n_idx // 16],
                            in_=d_idx[:, run_base * 8:
                                      run_base * 8 + n_idx // 16])
                        dm_t = mpool.tile([128, KRMAX], bf, tag="dm")
                        nc.sync.dma_start(
                            out=dm_t[:, :kr],
                            in_=d_dmv[:, run_base:run_base + kr])
                        gt = gpool.tile([128, KRMAX * 128], bf, tag="g")
                        for o in range(0, n_idx, GCALL):
                            nn_ = min(GCALL, n_idx - o)
                            nc.gpsimd.dma_gather(
                                out_ap=gt[:, o:o + nn_]
                                    .rearrange("p (k h) -> p k h", h=H),
                                in_ap=h1tab2[b * BLK2:(b + 1) * BLK2, :],
                                idxs_ap=it[:, o // 16:(o + nn_) // 16],
                                num_idxs=nn_,
                                num_idxs_reg=nn_,
                                elem_size=H,
                            )
                        oh_t = ohpool.tile([128, KRMAX * 128], bf, tag="oh")
                        for bi, k0 in enumerate(range(0, kr, 16)):
                            nb_ = min(16, kr - k0)
                            eng = nc.vector
                            eng.tensor_tensor(
                                out=oh_t[:, k0 * 128:(k0 + nb_) * 128],
                                in0=dm_t[:, k0:k0 + nb_]
                                    .to_broadcast([128, nb_, 128]),
                                in1=iotab[:].rearrange("p (k s) -> p k s", k=1)
                                    .to_broadcast([128, nb_, 128]),
                                op=ALU.is_equal,
                            )
                        ps = pspool4a.tile([128, GS * 128], f32, space="PSUM",
                                           tag="aggps", name="aggps")
                        ng_ = len(sup)
                        c0 = sup[0] * 128
                        if b > 0:
                            for z0 in range(0, ng_ * 128, 512):
                                zw = min(512, ng_ * 128 - z0)
                                nc.tensor.matmul(
                                    out=ps[:, z0:z0 + zw], lhsT=identb[:],
                                    rhs=agg2acc[:, c0 + z0:c0 + z0 + zw],
                                    start=True, stop=False)
                        for gi, g in enumerate(sup):
                            for kk in range(Kf[g, b]):
                                chunk = int(chunk_base[g, b]) + kk
                                off = (chunk - run_base) * 128
                                nc.tensor.matmul(
                                    out=ps[:, gi * 128:(gi + 1) * 128],
                                    lhsT=oh_t[:, off:off + 128],
                                    rhs=gt[:, off:off + H],
                                    start=(b == 0 and kk == 0),
                                    stop=(b == NB - 1 and kk == Kf[g, b] - 1))
                        if b != NB - 1:
                            nc.scalar.activation(
                                agg2acc[:, c0:c0 + ng_ * 128],
                                ps[:, :ng_ * 128], AF.Copy)
                            continue
                        for gi, g in enumerate(sup):
                            w = 128 if g < NG - 1 else SH - 128 * (NG - 1)
                            aggn = tpool.tile([128, H], bf, tag="aggn")
                            nc.scalar.activation(
                                aggn[:w, :], ps[:w, gi * 128:(gi + 1) * 128],
                                AF.Copy, scale=recrow[:w, g:g + 1])
                            pst = pspool4b.tile([128, 128], bf, space="PSUM",
                                                tag="tr")
                            nc.tensor.transpose(out=pst[:, :w],
                                                in_=aggn[:w, :],
                                                identity=identb[:w, :w])
                            aggnTc = tpool.tile([H, 128], bf, tag="aggnTc")
                            nc.scalar.activation(aggnTc[:, :w], pst[:, :w],
                                                 AF.Copy)
                            ps2 = pspool4b.tile([H, 128], f32, space="PSUM",
                                                tag="h2")
                            nc.tensor.matmul(out=ps2[:, :w], lhsT=w_s1[:],
                                             rhs=h1Tl[:, g * 128:g * 128 + w],
                                             start=True, stop=False)
                            nc.tensor.matmul(out=ps2[:, :w], lhsT=w_n1[:],
                                             rhs=aggnTc[:, :w], start=False,
                                             stop=True)
                            nc.scalar.activation(h2T[:, g * 128:g * 128 + w],
                                                 ps2[:, :w], AF.Relu,
                                                 bias=b1c[:, 0:1])

            # ---------------- Phase 5: LSTM ------------------------------
            with (
                tc.tile_pool(name="p5s", bufs=6) as spool,
                tc.tile_pool(name="p5ps", bufs=4, space="PSUM") as pspool5,
            ):
                hprev = [None, None]
                cprev = [None, None]

                def lstm_cell(t, l, xin):
                    ps = pspool5.tile([H, 4 * BL], f32, space="PSUM",
                                      tag=f"g{l}")
                    nc.tensor.matmul(out=ps[:], lhsT=b_s4[l][:],
                                     rhs=sel4[:], start=True, stop=False)
                    for q in range(4):
                        nc.tensor.matmul(
                            out=ps[:, q * BL:(q + 1) * BL],
                            lhsT=w_i[l][:, q * H:(q + 1) * H],
                            rhs=xin, start=False, stop=(t == 0))
                    if t > 0:
                        for q in range(4):
                            nc.tensor.matmul(
                                out=ps[:, q * BL:(q + 1) * BL],
                                lhsT=w_h[l][:, q * H:(q + 1) * H],
                                rhs=hprev[l][:], start=False, stop=True)
                    # gate cols: g [0:125], i [125:250], f [250:375],
                    # o [375:500]
                    tg = spool.tile([H, BL], f32, tag=f"tg{l}")
                    nc.scalar.activation(tg[:], ps[:, 0:BL], AF.Tanh)
                    sig = spool.tile([H, 2 * BL], f32, tag=f"sig{l}")
                    nc.scalar.activation(sig[:], ps[:, BL:3 * BL], AF.Sigmoid)
                    cnew = spool.tile([H, BL], f32, tag=f"c{l}")
                    if t > 0:
                        nc.gpsimd.tensor_tensor(out=cnew[:],
                                                in0=sig[:, BL:2 * BL],
                                                in1=cprev[l][:], op=ALU.mult)
                        t1 = spool.tile([H, BL], f32, tag=f"t1{l}")
                        nc.vector.tensor_tensor(out=t1[:], in0=sig[:, 0:BL],
                                                in1=tg[:], op=ALU.mult)
                        nc.vector.tensor_tensor(out=cnew[:], in0=cnew[:],
                                                in1=t1[:], op=ALU.add)
                    else:
                        nc.vector.tensor_tensor(out=cnew[:], in0=sig[:, 0:BL],
                                                in1=tg[:], op=ALU.mult)
                    sgo = spool.tile([H, BL], f32, tag=f"so{l}")
                    nc.scalar.activation(sgo[:], ps[:, 3 * BL:4 * BL],
                                         AF.Sigmoid)
                    tc_ = spool.tile([H, BL], f32, tag=f"tc{l}")
                    nc.scalar.activation(tc_[:], cnew[:], AF.Tanh)
                    hnew = spool.tile([H, BL], bf, tag=f"h{l}")
                    nc.vector.tensor_tensor(out=hnew[:], in0=sgo[:],
                                            in1=tc_[:], op=ALU.mult)
                    cprev[l] = cnew
                    hprev[l] = hnew
                    return hnew

                # software pipeline: layer 0 runs two steps ahead of
                # layer 1 so its activations fill layer-0's recurrence gaps
                import concourse.mybir as _mb
                h0q = [lstm_cell(0, 0, h2T[:, 0:BL]),
                       lstm_cell(1, 0, h2T[:, BL:2 * BL])]
                for t in range(T):
                    if t + 2 < T:
                        h0q.append(lstm_cell(t + 2, 0,
                                   h2T[:, (t + 2) * BL:(t + 3) * BL]))
                    h1c = lstm_cell(t, 1, h0q[t][:])
                    nc.vector.tensor_reduce(
                        out=pooledT[:, t:t + 1], in_=h1c[:],
                        axis=_mb.AxisListType.X, op=ALU.add)

            # ---------------- Phase 6: head ------------------------------
            with (
                tc.tile_pool(name="p6", bufs=1) as hp,
                tc.tile_pool(name="p6ps", bufs=1, space="PSUM") as psp,
            ):
                pooledb = hp.tile([H, B_GRAPHS], bf)
                nc.vector.tensor_copy(out=pooledb[:], in_=pooledT[:])
                nc.sync.dma_start(out=ccr_in[:], in_=pooledb[:])
                nc.gpsimd.collective_compute(
                    "AllGather", mybir.AluOpType.bypass,
                    replica_groups=[list(range(NCORES))],
                    ins=[ccr_in.opt()], outs=[ccr_out.opt()],
                )
                prT8 = hp.tile([H, NCORES * B_GRAPHS], bf)
                nc.sync.dma_start(
                    out=prT8[:].rearrange("h (r b) -> h r b", r=NCORES),
                    in_=ccr_out[:].rearrange("(r h) b -> h r b", h=H))
                prT = hp.tile([H, B_GRAPHS], f32)
                nc.vector.tensor_tensor(
                    out=prT[:], in0=prT8[:, 0:B_GRAPHS],
                    in1=prT8[:, B_GRAPHS:2 * B_GRAPHS], op=ALU.add)
                for r in range(2, NCORES):
                    nc.vector.tensor_tensor(
                        out=prT[:], in0=prT[:],
                        in1=prT8[:, r * B_GRAPHS:(r + 1) * B_GRAPHS],
                        op=ALU.add)
                psl = psp.tile([OUT, B_GRAPHS], f32, space="PSUM", tag="lg")
                nc.tensor.matmul(out=psl[:], lhsT=w_o[:], rhs=prT[:],
                                 start=True, stop=True)
                lg = hp.tile([OUT, B_GRAPHS], f32)
                nc.vector.tensor_scalar(out=lg[:], in0=psl[:],
                                        scalar1=b_o[:, 0:1], scalar2=None,
                                        op0=ALU.add)
                pst = psp.tile([B_GRAPHS, OUT], f32, space="PSUM", tag="lgt")
                nc.tensor.transpose(out=pst[:], in_=lg[:],
                                    identity=identf[0:OUT, 0:OUT])
                z = hp.tile([B_GRAPHS, OUT], f32)
                nc.vector.tensor_copy(out=z[:], in_=pst[:])
                m = hp.tile([B_GRAPHS, 1], f32)
                nc.vector.tensor_reduce(out=m[:], in_=z[:],
                                        axis=mybir.AxisListType.X, op=ALU.max)
                negm = hp.tile([B_GRAPHS, 1], f32)
                nc.vector.tensor_scalar(out=negm[:], in0=m[:], scalar1=-1.0,
                                        scalar2=None, op0=ALU.mult)
                e = hp.tile([B_GRAPHS, OUT], f32)
                se = hp.tile([B_GRAPHS, 1], f32)
                nc.scalar.activation(e[:], z[:], AF.Exp, bias=negm[:, 0:1],
                                     accum_out=se[:])
                ls = hp.tile([B_GRAPHS, 1], f32)
                nc.scalar.activation(ls[:], se[:], AF.Ln)
                o_sb = hp.tile([B_GRAPHS, OUT], f32)
                nc.vector.tensor_scalar(out=o_sb[:], in0=z[:],
                                        scalar1=m[:, 0:1], scalar2=ls[:, 0:1],
                                        op0=ALU.subtract, op1=ALU.subtract)
                nc.sync.dma_start(out=d_out[:], in_=o_sb[:])

    nc.compile()
    return nc


# --------------------------------------------------------------------------
# PJRT runner (built once, reused across calls)
# --------------------------------------------------------------------------
class _Runner:
    def __init__(self, nc, n_cores):
        import jax
        import concourse.mybir as mybir
        from jax.sharding import Mesh, PartitionSpec
        from jax.experimental.shard_map import shard_map
        from concourse.bass2jax import (
            _bass_exec_p, install_neuronx_cc_hook, partition_id_tensor)

        install_neuronx_cc_hook()
        self.n_cores = n_cores
        in_names, out_names, out_avals, zero_outs = [], [], [], []
        pname = nc.partition_id_tensor.name if nc.partition_id_tensor else None
        for alloc in nc.m.functions[0].allocations:
            if not isinstance(alloc, mybir.MemoryLocationSet):
                continue
            name = alloc.memorylocations[0].name
            if alloc.kind == "ExternalInput":
                if name != pname:
                    in_names.append(name)
            elif alloc.kind == "ExternalOutput":
                shape = tuple(alloc.tensor_shape)
                dtype = mybir.dt.np(alloc.dtype)
                out_names.append(name)
                out_avals.append(jax.core.ShapedArray(shape, dtype))
                zero_outs.append(np.zeros(shape, dtype))
        self.in_names, self.out_names = in_names, out_names
        self.out_avals, self.zero_outs = out_avals, zero_outs
        n_params, n_outs = len(in_names), len(out_names)
        all_in = list(in_names) + list(out_names) + ([pname] if pname else [])

        def _body(*args):
            operands = list(args)
            if pname is not None:
                operands.append(partition_id_tensor())
            return tuple(_bass_exec_p.bind(
                *operands, out_avals=tuple(out_avals),
                in_names=tuple(all_in), out_names=tuple(out_names),
                lowering_input_output_aliases=(),
                sim_require_finite=True, sim_require_nnan=True, nc=nc))

        devices = jax.devices()[:n_cores]
        mesh = Mesh(np.asarray(devices), ("core",))
        self._jax = jax
        self.sharded = jax.jit(
            shard_map(_body, mesh=mesh,
                      in_specs=(PartitionSpec("core"),) * (n_params + n_outs),
                      out_specs=(PartitionSpec("core"),) * n_outs,
                      check_rep=False),
            donate_argnums=tuple(range(n_params, n_params + n_outs)),
            keep_unused=True)

    def concat_inputs(self, in_maps):
        return [np.concatenate([np.asarray(m[nm]) for m in in_maps], axis=0)
                for nm in self.in_names]

    def run(self, concat_in):
        zeros = [np.zeros((self.n_cores * z.shape[0], *z.shape[1:]), z.dtype)
                 for z in self.zero_outs]
        out = self.sharded(*concat_in, *zeros)
        self._jax.block_until_ready(out)
        return out

    def split(self, out_arrs):
        return [{nm: np.asarray(out_arrs[i]).reshape(
            self.n_cores, *self.out_avals[i].shape)[c]
            for i, nm in enumerate(self.out_names)}
            for c in range(self.n_cores)]


def kernel(**inputs):
    shared, percore, meta, K, gmeta, NCH, chunk_base, L = _host_prep(inputs)
    if meta not in _BUILT:
        nc = _build_nc(K, gmeta, NCH, chunk_base, L)
        _BUILT[meta] = (nc, _Runner(nc, NCORES))
    nc, runner = _BUILT[meta]
    in_maps = [dict(shared, **percore[c]) for c in range(NCORES)]
    ci = runner.concat_inputs(in_maps)
    outs = runner.split(runner.run(ci))
    return np.asarray(outs[0]["out"], np.float32)


# revision 45
# speedup vs baseline: 1.0064x; 1.0064x over previous
"""DCRNN kernel for 8 Trainium2 NeuronCores (Bass/Tile).

Graph/data-parallel sharding (per hint): nodes permuted so core c owns
batch-lanes [c*125,(c+1)*125) of every graph; edges partitioned by dst shard.
conv1 aggregation uses a host-prepared dst-aligned layered table (layer k =
x4 of the k-th edge of each dst) so on-device it is a stack of wide DVE adds
— no gather, no one-hot.  The normalized 4-wide agg is AllGathered (tiny);
every core recomputes full h1 and writes a p-major bf16 h1 table (12.5KB DMA
descriptors).  conv2 buckets edges by (dst-group, src core-pair block) with
cross-core-uniform chunk counts; one dma_gather per (supergroup, block) run
(~5k indices amortizes SWDGE fixed cost) + one-hot matmul scatter in PSUM.
LSTM runs transposed-gates (batch 125/core) with bf16 weights, all 4 gates
in one PSUM tile with DMA-preloaded biases (3 activations/step); global mean
pool via free-dim reduce + AllReduce.
"""
import numpy as np
import ml_dtypes

BF16 = ml_dtypes.bfloat16

N = 100000
NPG = 1000
B_GRAPHS = 100
H = 128
CIN = 3
OUT = 2
NCORES = 8
SH = 12500          # nodes per core
NG = 98             # dst groups of 128 per core (last group = 84 nodes)
SHPAD = NG * 128    # 12544
NB = 4              # src blocks (core pairs) for conv2 gather
BLK2 = 2 * SHPAD    # 25088 rows per block in h1tab2 (int16-indexable)
GS = 8              # dst groups per super-group (conv2)
GCALL = 896         # idxs per dma_gather call
QW = 392            # quad-packed layer width (128 partitions * 392 = 4*12544)
T = 100
BL = 125            # batch lanes per core

_BUILT = {}


# --------------------------------------------------------------------------
# host preprocessing
# --------------------------------------------------------------------------
def _perm():
    n = np.arange(N)
    c = (n % NPG) // BL
    return c * SH + (n // NPG) * BL + (n % NPG) % BL


def _host_prep(inputs):
    x = np.asarray(inputs["x"], np.float32)
    ei = np.asarray(inputs["edge_index"])
    src, dst = ei[0].astype(np.int64), ei[1].astype(np.int64)
    p = _perm()
    srcp = p[src]
    dstp = p[dst]

    deg = np.bincount(dstp, minlength=N).astype(np.float32)
    recip = 1.0 / np.maximum(deg, 1.0)
    L = int(deg.max())                      # layers for conv1 dst-aligned agg

    # x in perm order, padded with a ones column (bias via W row)
    inv = np.empty(N, np.int64)
    inv[p] = np.arange(N)
    x4 = np.zeros((N, 4), np.float32)
    x4[:, :CIN] = x[inv]
    x4[:, CIN] = 1.0
    x4T = np.ascontiguousarray(x4.T)        # [4, N]
    x4Tp = np.zeros((4, NCORES * SHPAD), np.float32)  # SHPAD-padded per rank
    for c in range(NCORES):
        x4Tp[:, c * SHPAD:c * SHPAD + SH] = x4T[:, c * SH:(c + 1) * SH]

    owner = dstp // SH
    Ldst = dstp - owner * SH                # within-shard dst position

    # conv1: dst-aligned layered tables.  rank_within_dst via stable sort.
    order_d = np.argsort(dstp, kind="stable")
    d_sorted = dstp[order_d]
    s_sorted = srcp[order_d]
    run_start = np.zeros(N, np.int64)
    run_start[1:] = np.cumsum(np.bincount(dstp, minlength=N))[:-1]
    rank_d = np.arange(len(d_sorted)) - run_start[d_sorted]

    # h1tab2 p-major row ids: node (c, s) -> c*SHPAD + (s%128)*98 + s//128
    row2 = (srcp // SH) * SHPAD + (srcp % SH % 128) * 98 + (srcp % SH) // 128

    # conv2 bucketing (per core): (dst group g, src block b = row2//BLK2)
    K = np.zeros((NG, NB), np.int64)
    per_core = []
    for c in range(NCORES):
        m = owner == c
        Lc = Ldst[m]
        g = Lc // 128
        slot = (Lc % 128).astype(np.float32)
        r2 = row2[m]
        b = r2 // BLK2
        s16 = (r2 % BLK2).astype(np.int16)
        key = (g * NB + b).astype(np.int64)
        order = np.argsort(key, kind="stable")
        cnt = np.bincount(key, minlength=NG * NB)
        per_core.append((s16[order], slot[order], key[order], cnt))
        K = np.maximum(K, ((cnt + 127) // 128).reshape(NG, NB))
    K = np.maximum(K, 1)

    # chunk layout: for sup: for b: for g in sup: K[g,b] chunks
    sups = [range(i, min(i + GS, NG)) for i in range(0, NG, GS)]
    chunk_base = np.zeros((NG, NB), np.int64)
    gmeta = []
    nch = 0
    for sup in sups:
        bruns = []
        for b in range(NB):
            run_base = nch
            for g in sup:
                chunk_base[g, b] = nch
                nch += K[g, b]
            bruns.append((b, run_base, nch - run_base))
        gmeta.append(bruns)
    NCH = nch
    NSL = NCH * 128

    percore = []
    base_of_key = chunk_base.reshape(-1) * 128
    # quad-pack map for conv1 layers: position s -> part (s//QW)*4+f, col s%QW
    pos = np.arange(SH)
    qa, qc = pos // QW, pos % QW
    for c in range(NCORES):
        s_sorted2, slot_sorted, key_sorted, cnt = per_core[c]
        run_st = np.concatenate([[0], np.cumsum(cnt)[:-1]])
        rank_within = np.arange(len(s_sorted2)) - run_st[key_sorted]
        posn = base_of_key[key_sorted] + rank_within
        idx_flat = np.zeros(NSL, np.int16)
        dm_flat = np.full(NSL, -1.0, np.float32)
        idx_flat[posn] = s_sorted2
        dm_flat[posn] = slot_sorted
        w = idx_flat.reshape(NSL // 16, 16).T
        r = np.ones(SHPAD, np.float32)
        r[:SH] = recip[c * SH:(c + 1) * SH]

        # conv1 layered table [128, L*QW] bf16 + recq [128, QW] f32
        mm = (d_sorted >= c * SH) & (d_sorted < (c + 1) * SH)
        ls = d_sorted[mm] - c * SH          # within-shard dst pos, sorted
        lr = rank_d[mm]                     # layer index per edge
        lsrc = s_sorted[mm]                 # src node (perm id)
        xlay = np.zeros((128, L * QW), np.float32)
        partk = qa[ls] * 4                  # base partition of quad
        colk = lr * QW + qc[ls]
        for f in range(4):
            xlay[partk + f, colk] = x4[lsrc, f]
        recq = np.zeros((128, QW), np.float32)
        for f in range(4):
            recq[qa * 4 + f, qc] = recip[c * SH:(c + 1) * SH]

        xl = np.zeros((4, SHPAD), np.float32)
        xl[:, :SH] = x4T[:, c * SH:(c + 1) * SH]
        percore.append({
            "idx16": np.ascontiguousarray(np.tile(w, (8, 1)).astype(np.int16)),
            "dmv": np.ascontiguousarray(dm_flat.reshape(NCH, 128).T
                                        .astype(BF16)),
            "recrow": np.ascontiguousarray(r.reshape(NG, 128).T
                                           .astype(np.float32)),
            "xlay": np.ascontiguousarray(xlay.astype(BF16)),
            "recq": np.ascontiguousarray(recq),
            "x4tloc": xl.astype(BF16),
        })

    Wcomb = np.zeros((8, H), np.float32)
    Wcomb[0:3] = np.asarray(inputs["W_self0"], np.float32)
    Wcomb[3] = np.asarray(inputs["b0"], np.float32)
    Wcomb[4:7] = np.asarray(inputs["W_nbr0"], np.float32)

    shared = {
        "x4T": x4Tp.astype(BF16),
        "wcomb": Wcomb.astype(BF16),
        "ws1": np.asarray(inputs["W_self1"], np.float32).astype(BF16),
        "wn1": np.asarray(inputs["W_nbr1"], np.float32).astype(BF16),
        "b1c": np.ascontiguousarray(
            np.asarray(inputs["b1"], np.float32).reshape(H, 1)),
        "wo": (np.asarray(inputs["W_out"], np.float32) / NPG)
            .astype(np.float32),
        "bo": np.ascontiguousarray(
            np.asarray(inputs["b_out"], np.float32).reshape(OUT, 1)),
    }
    for l in range(2):
        wi = np.asarray(inputs[f"Wih{l}"], np.float32)
        wh = np.asarray(inputs[f"Whh{l}"], np.float32)
        bs = (np.asarray(inputs[f"bih{l}"], np.float32)
              + np.asarray(inputs[f"bhh{l}"], np.float32))
        # gate order (g, i, f, o): tanh(g) starts while i/f/o matmuls
        # still run; one sigmoid covers cols [125:500]
        GQ = (2, 0, 1, 3)
        shared[f"wi{l}"] = np.ascontiguousarray(np.concatenate(
            [wi[q * H:(q + 1) * H].T for q in GQ], axis=1)).astype(BF16)
        shared[f"wh{l}"] = np.ascontiguousarray(np.concatenate(
            [wh[q * H:(q + 1) * H].T for q in GQ], axis=1)).astype(BF16)
        shared[f"bs4_{l}"] = np.ascontiguousarray(
            bs.reshape(4, H)[list(GQ)].astype(BF16))
    sel4 = np.zeros((4, 4 * BL), np.float32)
    for q in range(4):
        sel4[q, q * BL:(q + 1) * BL] = 1.0
    shared["sel4"] = sel4.astype(BF16)

    meta = (L,) + tuple(K.reshape(-1).tolist())
    return shared, percore, meta, K, gmeta, NCH, chunk_base, L


# --------------------------------------------------------------------------
# device program
# --------------------------------------------------------------------------
def _build_nc(K, gmeta, NCH, chunk_base, L):
    import concourse.bacc as bacc
    import concourse.mybir as mybir
    from concourse.tile import TileContext
    from concourse.masks import make_identity

    f32 = mybir.dt.float32
    bf = mybir.dt.bfloat16
    i16 = mybir.dt.int16
    AF = mybir.ActivationFunctionType
    ALU = mybir.AluOpType
    NSL = NCH * 128
    Kf = K.reshape(NG, NB)
    sups = [range(i, min(i + GS, NG)) for i in range(0, NG, GS)]

    nc = bacc.Bacc(None, target_bir_lowering=False)

    d_x4T = nc.dram_tensor("x4T", [4, NCORES * SHPAD], bf,
                           kind="ExternalInput")
    d_wcomb = nc.dram_tensor("wcomb", [8, H], bf, kind="ExternalInput")
    d_ws1 = nc.dram_tensor("ws1", [H, H], bf, kind="ExternalInput")
    d_wn1 = nc.dram_tensor("wn1", [H, H], bf, kind="ExternalInput")
    d_b1c = nc.dram_tensor("b1c", [H, 1], f32, kind="ExternalInput")
    d_wo = nc.dram_tensor("wo", [H, OUT], f32, kind="ExternalInput")
    d_bo = nc.dram_tensor("bo", [OUT, 1], f32, kind="ExternalInput")
    d_wi = [nc.dram_tensor(f"wi{l}", [H, 4 * H], bf, kind="ExternalInput")
            for l in range(2)]
    d_wh = [nc.dram_tensor(f"wh{l}", [H, 4 * H], bf, kind="ExternalInput")
            for l in range(2)]
    d_bs4 = [nc.dram_tensor(f"bs4_{l}", [4, H], bf, kind="ExternalInput")
             for l in range(2)]
    d_sel4 = nc.dram_tensor("sel4", [4, 4 * BL], bf, kind="ExternalInput")
    d_idx = nc.dram_tensor("idx16", [128, NSL // 16], i16, kind="ExternalInput")
    d_dmv = nc.dram_tensor("dmv", [128, NCH], bf, kind="ExternalInput")
    d_recrow = nc.dram_tensor("recrow", [128, NG], f32, kind="ExternalInput")
    d_xlay = nc.dram_tensor("xlay", [128, L * QW], bf, kind="ExternalInput")
    d_recq = nc.dram_tensor("recq", [128, QW], f32, kind="ExternalInput")
    d_x4tloc = nc.dram_tensor("x4tloc", [4, SHPAD], bf, kind="ExternalInput")
    d_out = nc.dram_tensor("out", [B_GRAPHS, OUT], f32, kind="ExternalOutput")

    with TileContext(nc) as tc:
        with (
            tc.tile_pool(name="dram", bufs=1, space="DRAM") as dramp,
            tc.tile_pool(name="persist", bufs=1) as pers,
        ):
            h1tab2 = dramp.tile([NCORES * SHPAD, H], bf)
            cc_in = dramp.tile([4, SHPAD], bf)
            cc_out = dramp.tile([4 * NCORES, SHPAD], bf, addr_space="Shared")
            ccr_in = dramp.tile([H, B_GRAPHS], bf)
            ccr_out = dramp.tile([NCORES * H, B_GRAPHS], bf,
                                 addr_space="Shared")

            h2T = pers.tile([H, SHPAD], bf)
            h1Tl = pers.tile([H, SHPAD], bf)
            w_comb = pers.tile([8, H], bf)
            w_s1 = pers.tile([H, H], bf)
            w_n1 = pers.tile([H, H], bf)
            b1c = pers.tile([H, 1], f32)
            w_i = [pers.tile([H, 4 * H], bf, name=f"w_i{l}") for l in range(2)]
            w_h = [pers.tile([H, 4 * H], bf, name=f"w_h{l}") for l in range(2)]
            b_s4 = [pers.tile([4, H], bf, name=f"b_s4{l}") for l in range(2)]
            sel4 = pers.tile([4, 4 * BL], bf)
            w_o = pers.tile([H, OUT], f32)
            b_o = pers.tile([OUT, 1], f32)
            recrow = pers.tile([128, NG], f32)
            iotab = pers.tile([128, 128], bf)
            identb = pers.tile([128, 128], bf)
            identf = pers.tile([128, 128], f32)
            pooledT = pers.tile([H, B_GRAPHS], f32)

            nc.sync.dma_start(out=w_comb[:], in_=d_wcomb[:])
            nc.sync.dma_start(out=w_s1[:], in_=d_ws1[:])
            nc.sync.dma_start(out=w_n1[:], in_=d_wn1[:])
            nc.sync.dma_start(out=b1c[:], in_=d_b1c[:])
            for l in range(2):
                nc.sync.dma_start(out=w_i[l][:], in_=d_wi[l][:])
                nc.sync.dma_start(out=w_h[l][:], in_=d_wh[l][:])
                nc.sync.dma_start(out=b_s4[l][:], in_=d_bs4[l][:])
            nc.sync.dma_start(out=sel4[:], in_=d_sel4[:])
            nc.sync.dma_start(out=w_o[:], in_=d_wo[:])
            nc.sync.dma_start(out=b_o[:], in_=d_bo[:])
            nc.sync.dma_start(out=recrow[:], in_=d_recrow[:])
            iota_i = pers.tile([128, 128], mybir.dt.int32)
            nc.gpsimd.iota(iota_i[:], pattern=[[1, 128]], base=0,
                           channel_multiplier=0)
            nc.vector.tensor_copy(out=iotab[:], in_=iota_i[:])
            make_identity(nc, identf[:])
            nc.vector.tensor_copy(out=identb[:], in_=identf[:])

            # ---------------- Phase 1: conv1 layered aggregation ----------
            with (
                tc.tile_pool(name="p1l", bufs=2) as lpool,
                tc.tile_pool(name="p1a", bufs=1) as apool,
                tc.tile_pool(name="p1ps", bufs=1, space="PSUM") as pspool1,
            ):
                recq = apool.tile([128, QW], f32)
                nc.sync.dma_start(out=recq[:], in_=d_recq[:])
                acc = pspool1.tile([128, QW], f32, space="PSUM", tag="acc")
                LC = 8                      # layers per DMA chunk
                for k0 in range(0, L, LC):
                    nl = min(LC, L - k0)
                    lt = lpool.tile([128, LC * QW], bf, tag="lay")
                    nc.sync.dma_start(
                        out=lt[:, :nl * QW],
                        in_=d_xlay[:, k0 * QW:(k0 + nl) * QW])
                    for k in range(nl):
                        nc.tensor.matmul(
                            out=acc[:], lhsT=identb[:],
                            rhs=lt[:, k * QW:(k + 1) * QW],
                            start=(k0 + k == 0), stop=(k0 + k == L - 1))
                accb = apool.tile([128, QW], bf)
                nc.vector.tensor_tensor(out=accb[:], in0=acc[:],
                                        in1=recq[:], op=ALU.mult)
                # unpack quad layout [(a f), c] -> agg4T [f, (a c)] via a
                # DRAM bounce (a 2-level partition split is not a valid
                # SBUF access pattern, but is a plain strided DRAM AP)
                qscr = dramp.tile([128, QW], bf)
                nc.sync.dma_start(out=qscr[:], in_=accb[:])
                nc.sync.dma_start(
                    out=cc_in[:].rearrange("f (a c) -> f a c", a=32),
                    in_=qscr[:].rearrange("(a f) c -> f a c", f=4))

            nc.gpsimd.collective_compute(
                "AllGather", mybir.AluOpType.bypass,
                replica_groups=[list(range(NCORES))],
                ins=[cc_in.opt()], outs=[cc_out.opt()],
            )

            # -------- Phase 3: recompute h1 (all ranks), p-major table ----
            with (
                tc.tile_pool(name="p3xa", bufs=2) as xapool,
                tc.tile_pool(name="p3x1", bufs=1) as xapool1,
                tc.tile_pool(name="p3h", bufs=2) as hpool,
                tc.tile_pool(name="p3ps", bufs=3, space="PSUM") as pspool3,
            ):
                # own-shard transposed h1 — needs only local agg4T, so it
                # overlaps the AllGather above
                xal = xapool1.tile([8, SHPAD], bf, tag="xal")
                nc.sync.dma_start(out=xal[0:4, :], in_=d_x4tloc[:])
                nc.sync.dma_start(out=xal[4:8, :], in_=cc_in[:])
                for j0 in range(0, NG, 4):
                    nj = min(4, NG - j0)
                    ps = pspool3.tile([H, 4 * 128], f32, space="PSUM",
                                      tag="psl")
                    for j in range(j0, j0 + nj):
                        nc.tensor.matmul(
                            out=ps[:, (j - j0) * 128:(j - j0 + 1) * 128],
                            lhsT=w_comb[:],
                            rhs=xal[:, j * 128:(j + 1) * 128],
                            start=True, stop=True)
                    nc.scalar.activation(
                        h1Tl[:, j0 * 128:(j0 + nj) * 128],
                        ps[:, :nj * 128], AF.Relu)

                for r in range(NCORES):
                    xa = xapool.tile([8, SHPAD], bf, tag="xa")
                    nc.sync.dma_start(out=xa[0:4, :],
                                      in_=d_x4T[:, r * SHPAD:(r + 1) * SHPAD])
                    nc.sync.dma_start(out=xa[4:8, :],
                                      in_=cc_out[4 * r:4 * r + 4, :])
                    for half in range(2):
                        h0 = half * 49
                        hb = hpool.tile([128, 49 * 128], bf, tag="hrow")
                        for bi, j0 in enumerate(range(h0, h0 + 49, 4)):
                            nj = min(4, h0 + 49 - j0)
                            ps = pspool3.tile([128, 4 * H], f32, space="PSUM",
                                              tag="psr")
                            for j in range(j0, j0 + nj):
                                nc.tensor.matmul(
                                    out=ps[:, (j - j0) * H:(j - j0 + 1) * H],
                                    lhsT=xa[:, j * 128:(j + 1) * 128],
                                    rhs=w_comb[:], start=True, stop=True)
                            dst = hb[:, (j0 - h0) * 128:(j0 - h0 + nj) * 128]
                            if bi % 2 == 0:
                                nc.scalar.activation(dst, ps[:, :nj * H],
                                                     AF.Relu)
                            else:
                                nc.vector.tensor_scalar(
                                    out=dst, in0=ps[:, :nj * H], scalar1=0.0,
                                    scalar2=None, op0=ALU.max)
                        # p-major write: rows p*98+h0+(0..48), 12.5KB/desc
                        nc.sync.dma_start(
                            out=h1tab2[r * SHPAD:(r + 1) * SHPAD, :]
                                .rearrange("(p j) h -> p (j h)", p=128)
                                [:, h0 * 128:(h0 + 49) * 128],
                            in_=hb[:, :])

            # ---------------- Phase 4: conv2 -----------------------------
            with (
                tc.tile_pool(name="p4g", bufs=3) as gpool,
                tc.tile_pool(name="p4oh", bufs=2) as ohpool,
                tc.tile_pool(name="p4m", bufs=3) as mpool,
                tc.tile_pool(name="p4ps", bufs=2, space="PSUM") as pspool4a,
                tc.tile_pool(name="p4ps2", bufs=2, space="PSUM") as pspool4b,
                tc.tile_pool(name="p4t", bufs=3) as tpool,
            ):
                KRMAX = max(kr for bruns in gmeta for _, _, kr in bruns)
                agg2acc = pers.tile([128, NG * 128], bf)
                # b-major: block b's gathers depend only on h1 of cores
                # 2b/2b+1, overlapping conv2 with the tail of phase 3
                for b in range(NB):
                    for bruns, sup in zip(gmeta, sups):
                        _, run_base, kr = bruns[b]
                        n_idx = kr * 128
                        it = mpool.tile([128, (KRMAX * 128) // 16], i16,
                                        tag="idx")
                        nc.sync.dma_start(
                            out=it[:, :n_idx // 16],
                            in_=d_idx[:, run_base * 8:
                                      run_base * 8 + n_idx // 16])
                        dm_t = mpool.tile([128, KRMAX], bf, tag="dm")
                        nc.sync.dma_start(
                            out=dm_t[:, :kr],
                            in_=d_dmv[:, run_base:run_base + kr])
                        gt = gpool.tile([128, KRMAX * 128], bf, tag="g")
                        for o in range(0, n_idx, GCALL):
                            nn_ = min(GCALL, n_idx - o)
                            nc.gpsimd.dma_gather(
                                out_ap=gt[:, o:o + nn_]
                                    .rearrange("p (k h) -> p k h", h=H),
                                in_ap=h1tab2[b * BLK2:(b + 1) * BLK2, :],
                                idxs_ap=it[:, o // 16:(o + nn_) // 16],
                                num_idxs=nn_,
                                num_idxs_reg=nn_,
                                elem_size=H,
                            )
                        oh_t = ohpool.tile([128, KRMAX * 128], bf, tag="oh")
                        for bi, k0 in enumerate(range(0, kr, 16)):
                            nb_ = min(16, kr - k0)
                            eng = nc.vector
                            eng.tensor_tensor(
                                out=oh_t[:, k0 * 128:(k0 + nb_) * 128],
                                in0=dm_t[:, k0:k0 + nb_]
                                    .to_broadcast([128, nb_, 128]),
                                in1=iotab[:].rearrange("p (k s) -> p k s", k=1)
                                    .to_broadcast([128, nb_, 128]),
                                op=ALU.is_equal,
                            )
                        ps = pspool4a.tile([128, GS * 128], f32, space="PSUM",
                                           tag="aggps", name="aggps")
                        ng_ = len(sup)
                        c0 = sup[0] * 128
                        if b > 0:
                            for z0 in range(0, ng_ * 128, 512):
                                zw = min(512, ng_ * 128 - z0)
                                nc.tensor.matmul(
                                    out=ps[:, z0:z0 + zw], lhsT=identb[:],
                                    rhs=agg2acc[:, c0 + z0:c0 + z0 + zw],
                                    start=True, stop=False)
                        for gi, g in enumerate(sup):
                            for kk in range(Kf[g, b]):
                                chunk = int(chunk_base[g, b]) + kk
                                off = (chunk - run_base) * 128
                                nc.tensor.matmul(
                                    out=ps[:, gi * 128:(gi + 1) * 128],
                                    lhsT=oh_t[:, off:off + 128],
                                    rhs=gt[:, off:off + H],
                                    start=(b == 0 and kk == 0),
                                    stop=(b == NB - 1 and kk == Kf[g, b] - 1))
                        if b != NB - 1:
                            nc.scalar.activation(
                                agg2acc[:, c0:c0 + ng_ * 128],
                                ps[:, :ng_ * 128], AF.Copy)
                            continue
                        for gi, g in enumerate(sup):
                            w = 128 if g < NG - 1 else SH - 128 * (NG - 1)
                            aggn = tpool.tile([128, H], bf, tag="aggn")
                            nc.scalar.activation(
                                aggn[:w, :], ps[:w, gi * 128:(gi + 1) * 128],
                                AF.Copy, scale=recrow[:w, g:g + 1])
                            pst = pspool4b.tile([128, 128], bf, space="PSUM",
                                                tag="tr")
                            nc.tensor.transpose(out=pst[:, :w],
                                                in_=aggn[:w, :],
                                                identity=identb[:w, :w])
                            aggnTc = tpool.tile([H, 128], bf, tag="aggnTc")
                            nc.scalar.activation(aggnTc[:, :w], pst[:, :w],
                                                 AF.Copy)
                            ps2 = pspool4b.tile([H, 128], f32, space="PSUM",
                                                tag="h2")
                            nc.tensor.matmul(out=ps2[:, :w], lhsT=w_s1[:],
                                             rhs=h1Tl[:, g * 128:g * 128 + w],
                                             start=True, stop=False)
                            nc.tensor.matmul(out=ps2[:, :w], lhsT=w_n1[:],
                                             rhs=aggnTc[:, :w], start=False,
                                             stop=True)
                            nc.scalar.activation(h2T[:, g * 128:g * 128 + w],
                                                 ps2[:, :w], AF.Relu,
                                                 bias=b1c[:, 0:1])

            # ---------------- Phase 5: LSTM ------------------------------
            with (
                tc.tile_pool(name="p5s", bufs=6) as spool,
                tc.tile_pool(name="p5ps", bufs=4, space="PSUM") as pspool5,
            ):
                hprev = [None, None]
                cprev = [None, None]

                def lstm_cell(t, l, xin):
                    ps = pspool5.tile([H, 4 * BL], f32, space="PSUM",
                                      tag=f"g{l}")
                    nc.tensor.matmul(out=ps[:], lhsT=b_s4[l][:],
                                     rhs=sel4[:], start=True, stop=False)
                    for q in range(4):
                        nc.tensor.matmul(
                            out=ps[:, q * BL:(q + 1) * BL],
                            lhsT=w_i[l][:, q * H:(q + 1) * H],
                            rhs=xin, start=False, stop=(t == 0))
                    if t > 0:
                        for q in range(4):
                            nc.tensor.matmul(
                                out=ps[:, q * BL:(q + 1) * BL],
                                lhsT=w_h[l][:, q * H:(q + 1) * H],
                                rhs=hprev[l][:], start=False, stop=True)
                    # gate cols: g [0:125], i [125:250], f [250:375],
                    # o [375:500]
                    tg = spool.tile([H, BL], f32, tag=f"tg{l}")
                    nc.scalar.activation(tg[:], ps[:, 0:BL], AF.Tanh)
                    sig = spool.tile([H, 2 * BL], f32, tag=f"sig{l}")
                    nc.scalar.activation(sig[:], ps[:, BL:3 * BL], AF.Sigmoid)
                    cnew = spool.tile([H, BL], f32, tag=f"c{l}")
                    if t > 0:
                        nc.vector.tensor_tensor(out=cnew[:],
                                                in0=sig[:, BL:2 * BL],
                                                in1=cprev[l][:], op=ALU.mult)
                        t1 = spool.tile([H, BL], f32, tag=f"t1{l}")
                        nc.vector.tensor_tensor(out=t1[:], in0=sig[:, 0:BL],
                                                in1=tg[:], op=ALU.mult)
                        nc.vector.tensor_tensor(out=cnew[:], in0=cnew[:],
                                                in1=t1[:], op=ALU.add)
                    else:
                        nc.vector.tensor_tensor(out=cnew[:], in0=sig[:, 0:BL],
                                                in1=tg[:], op=ALU.mult)
                    sgo = spool.tile([H, BL], f32, tag=f"so{l}")
                    nc.scalar.activation(sgo[:], ps[:, 3 * BL:4 * BL],
                                         AF.Sigmoid)
                    tc_ = spool.tile([H, BL], f32, tag=f"tc{l}")
                    nc.scalar.activation(tc_[:], cnew[:], AF.Tanh)
                    hnew = spool.tile([H, BL], bf, tag=f"h{l}")
                    nc.vector.tensor_tensor(out=hnew[:], in0=sgo[:],
                                            in1=tc_[:], op=ALU.mult)
                    cprev[l] = cnew
                    hprev[l] = hnew
                    return hnew

                # software pipeline: layer 0 runs two steps ahead of
                # layer 1 so its activations fill layer-0's recurrence gaps
                import concourse.mybir as _mb
                h0q = [lstm_cell(0, 0, h2T[:, 0:BL]),
                       lstm_cell(1, 0, h2T[:, BL:2 * BL]),
                       lstm_cell(2, 0, h2T[:, 2 * BL:3 * BL])]
                for t in range(T):
                    if t + 3 < T:
                        h0q.append(lstm_cell(t + 3, 0,
                                   h2T[:, (t + 3) * BL:(t + 4) * BL]))
                    h1c = lstm_cell(t, 1, h0q[t][:])
                    nc.vector.tensor_reduce(
                        out=pooledT[:, t:t + 1], in_=h1c[:],
                        axis=_mb.AxisListType.X, op=ALU.add)

            # ---------------- Phase 6: head ------------------------------
            with (
                tc.tile_pool(name="p6", bufs=1) as hp,
                tc.tile_pool(name="p6ps", bufs=1, space="PSUM") as psp,
            ):
                pooledb = hp.tile([H, B_GRAPHS], bf)
                nc.vector.tensor_copy(out=pooledb[:], in_=pooledT[:])
                nc.sync.dma_start(out=ccr_in[:], in_=pooledb[:])
                nc.gpsimd.collective_compute(
                    "AllGather", mybir.AluOpType.bypass,
                    replica_groups=[list(range(NCORES))],
                    ins=[ccr_in.opt()], outs=[ccr_out.opt()],
                )
                prT8 = hp.tile([H, NCORES * B_GRAPHS], bf)
                nc.sync.dma_start(
                    out=prT8[:].rearrange("h (r b) -> h r b", r=NCORES),
                    in_=ccr_out[:].rearrange("(r h) b -> h r b", h=H))
                prT = hp.tile([H, B_GRAPHS], f32)
                nc.vector.tensor_tensor(
                    out=prT[:], in0=prT8[:, 0:B_GRAPHS],
                    in1=prT8[:, B_GRAPHS:2 * B_GRAPHS], op=ALU.add)
                for r in range(2, NCORES):
                    nc.vector.tensor_tensor(
                        out=prT[:], in0=prT[:],
                        in1=prT8[:, r * B_GRAPHS:(r + 1) * B_GRAPHS],
                        op=ALU.add)
                psl = psp.tile([OUT, B_GRAPHS], f32, space="PSUM", tag="lg")
                nc.tensor.matmul(out=psl[:], lhsT=w_o[:], rhs=prT[:],
                                 start=True, stop=True)
                lg = hp.tile([OUT, B_GRAPHS], f32)
                nc.vector.tensor_scalar(out=lg[:], in0=psl[:],
                                        scalar1=b_o[:, 0:1], scalar2=None,
                                        op0=ALU.add)
                pst = psp.tile([B_GRAPHS, OUT], f32, space="PSUM", tag="lgt")
                nc.tensor.transpose(out=pst[:], in_=lg[:],
                                    identity=identf[0:OUT, 0:OUT])
                z = hp.tile([B_GRAPHS, OUT], f32)
                nc.vector.tensor_copy(out=z[:], in_=pst[:])
                m = hp.tile([B_GRAPHS, 1], f32)
                nc.vector.tensor_reduce(out=m[:], in_=z[:],
                                        axis=mybir.AxisListType.X, op=ALU.max)
                negm = hp.tile([B_GRAPHS, 1], f32)
                nc.vector.tensor_scalar(out=negm[:], in0=m[:], scalar1=-1.0,
                                        scalar2=None, op0=ALU.mult)
                e = hp.tile([B_GRAPHS, OUT], f32)
                se = hp.tile([B_GRAPHS, 1], f32)
                nc.scalar.activation(e[:], z[:], AF.Exp, bias=negm[:, 0:1],
                                     accum_out=se[:])
                ls = hp.tile([B_GRAPHS, 1], f32)
                nc.scalar.activation(ls[:], se[:], AF.Ln)
                o_sb = hp.tile([B_GRAPHS, OUT], f32)
                nc.vector.tensor_scalar(out=o_sb[:], in0=z[:],
                                        scalar1=m[:, 0:1], scalar2=ls[:, 0:1],
                                        op0=ALU.subtract, op1=ALU.subtract)
                nc.sync.dma_start(out=d_out[:], in_=o_sb[:])

    nc.compile()
    return nc


# --------------------------------------------------------------------------
# PJRT runner (built once, reused across calls)
# --------------------------------------------------------------------------
class _Runner:
    def __init__(self, nc, n_cores):
        import jax
        import concourse.mybir as mybir
        from jax.sharding import Mesh, PartitionSpec
        from jax.experimental.shard_map import shard_map
        from concourse.bass2jax import (
            _bass_exec_p, install_neuronx_cc_hook, partition_id_tensor)

        install_neuronx_cc_hook()
        self.n_cores = n_cores
        in_names, out_names, out_avals, zero_outs = [], [], [], []
        pname = nc.partition_id_tensor.name if nc.partition_id_tensor else None
        for alloc in nc.m.functions[0].allocations:
            if not isinstance(alloc, mybir.MemoryLocationSet):
                continue
            name = alloc.memorylocations[0].name
            if alloc.kind == "ExternalInput":
                if name != pname:
                    in_names.append(name)
            elif alloc.kind == "ExternalOutput":
                shape = tuple(alloc.tensor_shape)
                dtype = mybir.dt.np(alloc.dtype)
                out_names.append(name)
                out_avals.append(jax.core.ShapedArray(shape, dtype))
                zero_outs.append(np.zeros(shape, dtype))
        self.in_names, self.out_names = in_names, out_names
        self.out_avals, self.zero_outs = out_avals, zero_outs
        n_params, n_outs = len(in_names), len(out_names)
        all_in = list(in_names) + list(out_names) + ([pname] if pname else [])

        def _body(*args):
            operands = list(args)
            if pname is not None:
                operands.append(partition_id_tensor())
            return tuple(_bass_exec_p.bind(
                *operands, out_avals=tuple(out_avals),
                in_names=tuple(all_in), out_names=tuple(out_names),
                lowering_input_output_aliases=(),
                sim_require_finite=True, sim_require_nnan=True, nc=nc))

        devices = jax.devices()[:n_cores]
        mesh = Mesh(np.asarray(devices), ("core",))
        self._jax = jax
        self.sharded = jax.jit(
            shard_map(_body, mesh=mesh,
                      in_specs=(PartitionSpec("core"),) * (n_params + n_outs),
                      out_specs=(PartitionSpec("core"),) * n_outs,
                      check_rep=False),
            donate_argnums=tuple(range(n_params, n_params + n_outs)),
            keep_unused=True)

    def concat_inputs(self, in_maps):
        return [np.concatenate([np.asarray(m[nm]) for m in in_maps], axis=0)
                for nm in self.in_names]

    def run(self, concat_in):
        zeros = [np.zeros((self.n_cores * z.shape[0], *z.shape[1:]), z.dtype)
                 for z in self.zero_outs]
        out = self.sharded(*concat_in, *zeros)
        self._jax.block_until_ready(out)
        return out

    def split(self, out_arrs):
        return [{nm: np.asarray(out_arrs[i]).reshape(
            self.n_cores, *self.out_avals[i].shape)[c]
            for i, nm in enumerate(self.out_names)}
            for c in range(self.n_cores)]


def kernel(**inputs):
    shared, percore, meta, K, gmeta, NCH, chunk_base, L = _host_prep(inputs)
    if meta not in _BUILT:
        nc = _build_nc(K, gmeta, NCH, chunk_base, L)
        _BUILT[meta] = (nc, _Runner(nc, NCORES))
    nc, runner = _BUILT[meta]
    in_maps = [dict(shared, **percore[c]) for c in range(NCORES)]
    ci = runner.concat_inputs(in_maps)
    outs = runner.split(runner.run(ci))
    return np.asarray(outs[0]["out"], np.float32)


# revision 46
# speedup vs baseline: 1.0073x; 1.0009x over previous
"""DCRNN kernel for 8 Trainium2 NeuronCores (Bass/Tile).

Graph/data-parallel sharding (per hint): nodes permuted so core c owns
batch-lanes [c*125,(c+1)*125) of every graph; edges partitioned by dst shard.
conv1 aggregation uses a host-prepared dst-aligned layered table (layer k =
x4 of the k-th edge of each dst) so on-device it is a stack of wide DVE adds
— no gather, no one-hot.  The normalized 4-wide agg is AllGathered (tiny);
every core recomputes full h1 and writes a p-major bf16 h1 table (12.5KB DMA
descriptors).  conv2 buckets edges by (dst-group, src core-pair block) with
cross-core-uniform chunk counts; one dma_gather per (supergroup, block) run
(~5k indices amortizes SWDGE fixed cost) + one-hot matmul scatter in PSUM.
LSTM runs transposed-gates (batch 125/core) with bf16 weights, all 4 gates
in one PSUM tile with DMA-preloaded biases (3 activations/step); global mean
pool via free-dim reduce + AllReduce.
"""
import numpy as np
import ml_dtypes

BF16 = ml_dtypes.bfloat16

N = 100000
NPG = 1000
B_GRAPHS = 100
H = 128
CIN = 3
OUT = 2
NCORES = 8
SH = 12500          # nodes per core
NG = 98             # dst groups of 128 per core (last group = 84 nodes)
SHPAD = NG * 128    # 12544
NB = 4              # src blocks (core pairs) for conv2 gather
BLK2 = 2 * SHPAD    # 25088 rows per block in h1tab2 (int16-indexable)
GS = 8              # dst groups per super-group (conv2)
GCALL = 896         # idxs per dma_gather call
QW = 392            # quad-packed layer width (128 partitions * 392 = 4*12544)
T = 100
BL = 125            # batch lanes per core

_BUILT = {}


# --------------------------------------------------------------------------
# host preprocessing
# --------------------------------------------------------------------------
def _perm():
    n = np.arange(N)
    c = (n % NPG) // BL
    return c * SH + (n // NPG) * BL + (n % NPG) % BL


def _host_prep(inputs):
    x = np.asarray(inputs["x"], np.float32)
    ei = np.asarray(inputs["edge_index"])
    src, dst = ei[0].astype(np.int64), ei[1].astype(np.int64)
    p = _perm()
    srcp = p[src]
    dstp = p[dst]

    deg = np.bincount(dstp, minlength=N).astype(np.float32)
    recip = 1.0 / np.maximum(deg, 1.0)
    L = int(deg.max())                      # layers for conv1 dst-aligned agg

    # x in perm order, padded with a ones column (bias via W row)
    inv = np.empty(N, np.int64)
    inv[p] = np.arange(N)
    x4 = np.zeros((N, 4), np.float32)
    x4[:, :CIN] = x[inv]
    x4[:, CIN] = 1.0
    x4T = np.ascontiguousarray(x4.T)        # [4, N]
    x4Tp = np.zeros((4, NCORES * SHPAD), np.float32)  # SHPAD-padded per rank
    for c in range(NCORES):
        x4Tp[:, c * SHPAD:c * SHPAD + SH] = x4T[:, c * SH:(c + 1) * SH]

    owner = dstp // SH
    Ldst = dstp - owner * SH                # within-shard dst position

    # conv1: dst-aligned layered tables.  rank_within_dst via stable sort.
    order_d = np.argsort(dstp, kind="stable")
    d_sorted = dstp[order_d]
    s_sorted = srcp[order_d]
    run_start = np.zeros(N, np.int64)
    run_start[1:] = np.cumsum(np.bincount(dstp, minlength=N))[:-1]
    rank_d = np.arange(len(d_sorted)) - run_start[d_sorted]

    # h1tab2 p-major row ids: node (c, s) -> c*SHPAD + (s%128)*98 + s//128
    row2 = (srcp // SH) * SHPAD + (srcp % SH % 128) * 98 + (srcp % SH) // 128

    # conv2 bucketing (per core): (dst group g, src block b = row2//BLK2)
    K = np.zeros((NG, NB), np.int64)
    per_core = []
    for c in range(NCORES):
        m = owner == c
        Lc = Ldst[m]
        g = Lc // 128
        slot = (Lc % 128).astype(np.float32)
        r2 = row2[m]
        b = r2 // BLK2
        s16 = (r2 % BLK2).astype(np.int16)
        key = (g * NB + b).astype(np.int64)
        order = np.argsort(key, kind="stable")
        cnt = np.bincount(key, minlength=NG * NB)
        per_core.append((s16[order], slot[order], key[order], cnt))
        K = np.maximum(K, ((cnt + 127) // 128).reshape(NG, NB))
    K = np.maximum(K, 1)

    # chunk layout: for sup: for b: for g in sup: K[g,b] chunks
    sups = [range(i, min(i + GS, NG)) for i in range(0, NG, GS)]
    chunk_base = np.zeros((NG, NB), np.int64)
    gmeta = []
    nch = 0
    for sup in sups:
        bruns = []
        for b in range(NB):
            run_base = nch
            for g in sup:
                chunk_base[g, b] = nch
                nch += K[g, b]
            bruns.append((b, run_base, nch - run_base))
        gmeta.append(bruns)
    NCH = nch
    NSL = NCH * 128

    percore = []
    base_of_key = chunk_base.reshape(-1) * 128
    # quad-pack map for conv1 layers: position s -> part (s//QW)*4+f, col s%QW
    pos = np.arange(SH)
    qa, qc = pos // QW, pos % QW
    for c in range(NCORES):
        s_sorted2, slot_sorted, key_sorted, cnt = per_core[c]
        run_st = np.concatenate([[0], np.cumsum(cnt)[:-1]])
        rank_within = np.arange(len(s_sorted2)) - run_st[key_sorted]
        posn = base_of_key[key_sorted] + rank_within
        idx_flat = np.zeros(NSL, np.int16)
        dm_flat = np.full(NSL, -1.0, np.float32)
        idx_flat[posn] = s_sorted2
        dm_flat[posn] = slot_sorted
        w = idx_flat.reshape(NSL // 16, 16).T
        r = np.ones(SHPAD, np.float32)
        r[:SH] = recip[c * SH:(c + 1) * SH]

        # conv1 layered table [128, L*QW] bf16 + recq [128, QW] f32
        mm = (d_sorted >= c * SH) & (d_sorted < (c + 1) * SH)
        ls = d_sorted[mm] - c * SH          # within-shard dst pos, sorted
        lr = rank_d[mm]                     # layer index per edge
        lsrc = s_sorted[mm]                 # src node (perm id)
        xlay = np.zeros((128, L * QW), np.float32)
        partk = qa[ls] * 4                  # base partition of quad
        colk = lr * QW + qc[ls]
        for f in range(4):
            xlay[partk + f, colk] = x4[lsrc, f]
        recq = np.zeros((128, QW), np.float32)
        for f in range(4):
            recq[qa * 4 + f, qc] = recip[c * SH:(c + 1) * SH]

        xl = np.zeros((4, SHPAD), np.float32)
        xl[:, :SH] = x4T[:, c * SH:(c + 1) * SH]
        percore.append({
            "idx16": np.ascontiguousarray(np.tile(w, (8, 1)).astype(np.int16)),
            "dmv": np.ascontiguousarray(dm_flat.reshape(NCH, 128).T
                                        .astype(BF16)),
            "recrow": np.ascontiguousarray(r.reshape(NG, 128).T
                                           .astype(np.float32)),
            "xlay": np.ascontiguousarray(xlay.astype(BF16)),
            "recq": np.ascontiguousarray(recq),
            "x4tloc": xl.astype(BF16),
        })

    Wcomb = np.zeros((8, H), np.float32)
    Wcomb[0:3] = np.asarray(inputs["W_self0"], np.float32)
    Wcomb[3] = np.asarray(inputs["b0"], np.float32)
    Wcomb[4:7] = np.asarray(inputs["W_nbr0"], np.float32)

    shared = {
        "x4T": x4Tp.astype(BF16),
        "wcomb": Wcomb.astype(BF16),
        "ws1": np.asarray(inputs["W_self1"], np.float32).astype(BF16),
        "wn1": np.asarray(inputs["W_nbr1"], np.float32).astype(BF16),
        "b1c": np.ascontiguousarray(
            np.asarray(inputs["b1"], np.float32).reshape(H, 1)),
        "wo": (np.asarray(inputs["W_out"], np.float32) / NPG)
            .astype(np.float32),
        "bo": np.ascontiguousarray(
            np.asarray(inputs["b_out"], np.float32).reshape(OUT, 1)),
    }
    for l in range(2):
        wi = np.asarray(inputs[f"Wih{l}"], np.float32)
        wh = np.asarray(inputs[f"Whh{l}"], np.float32)
        bs = (np.asarray(inputs[f"bih{l}"], np.float32)
              + np.asarray(inputs[f"bhh{l}"], np.float32))
        # gate order (g, i, f, o): tanh(g) starts while i/f/o matmuls
        # still run; one sigmoid covers cols [125:500]
        GQ = (2, 0, 1, 3)
        shared[f"wi{l}"] = np.ascontiguousarray(np.concatenate(
            [wi[q * H:(q + 1) * H].T for q in GQ], axis=1)).astype(BF16)
        shared[f"wh{l}"] = np.ascontiguousarray(np.concatenate(
            [wh[q * H:(q + 1) * H].T for q in GQ], axis=1)).astype(BF16)
        shared[f"bs4_{l}"] = np.ascontiguousarray(
            bs.reshape(4, H)[list(GQ)].astype(BF16))
    sel4 = np.zeros((4, 4 * BL), np.float32)
    for q in range(4):
        sel4[q, q * BL:(q + 1) * BL] = 1.0
    shared["sel4"] = sel4.astype(BF16)

    meta = (L,) + tuple(K.reshape(-1).tolist())
    return shared, percore, meta, K, gmeta, NCH, chunk_base, L


# --------------------------------------------------------------------------
# device program
# --------------------------------------------------------------------------
def _build_nc(K, gmeta, NCH, chunk_base, L):
    import concourse.bacc as bacc
    import concourse.mybir as mybir
    from concourse.tile import TileContext
    from concourse.masks import make_identity

    f32 = mybir.dt.float32
    bf = mybir.dt.bfloat16
    i16 = mybir.dt.int16
    AF = mybir.ActivationFunctionType
    ALU = mybir.AluOpType
    NSL = NCH * 128
    Kf = K.reshape(NG, NB)
    sups = [range(i, min(i + GS, NG)) for i in range(0, NG, GS)]

    nc = bacc.Bacc(None, target_bir_lowering=False)

    d_x4T = nc.dram_tensor("x4T", [4, NCORES * SHPAD], bf,
                           kind="ExternalInput")
    d_wcomb = nc.dram_tensor("wcomb", [8, H], bf, kind="ExternalInput")
    d_ws1 = nc.dram_tensor("ws1", [H, H], bf, kind="ExternalInput")
    d_wn1 = nc.dram_tensor("wn1", [H, H], bf, kind="ExternalInput")
    d_b1c = nc.dram_tensor("b1c", [H, 1], f32, kind="ExternalInput")
    d_wo = nc.dram_tensor("wo", [H, OUT], f32, kind="ExternalInput")
    d_bo = nc.dram_tensor("bo", [OUT, 1], f32, kind="ExternalInput")
    d_wi = [nc.dram_tensor(f"wi{l}", [H, 4 * H], bf, kind="ExternalInput")
            for l in range(2)]
    d_wh = [nc.dram_tensor(f"wh{l}", [H, 4 * H], bf, kind="ExternalInput")
            for l in range(2)]
    d_bs4 = [nc.dram_tensor(f"bs4_{l}", [4, H], bf, kind="ExternalInput")
             for l in range(2)]
    d_sel4 = nc.dram_tensor("sel4", [4, 4 * BL], bf, kind="ExternalInput")
    d_idx = nc.dram_tensor("idx16", [128, NSL // 16], i16, kind="ExternalInput")
    d_dmv = nc.dram_tensor("dmv", [128, NCH], bf, kind="ExternalInput")
    d_recrow = nc.dram_tensor("recrow", [128, NG], f32, kind="ExternalInput")
    d_xlay = nc.dram_tensor("xlay", [128, L * QW], bf, kind="ExternalInput")
    d_recq = nc.dram_tensor("recq", [128, QW], f32, kind="ExternalInput")
    d_x4tloc = nc.dram_tensor("x4tloc", [4, SHPAD], bf, kind="ExternalInput")
    d_out = nc.dram_tensor("out", [B_GRAPHS, OUT], f32, kind="ExternalOutput")

    with TileContext(nc) as tc:
        with (
            tc.tile_pool(name="dram", bufs=1, space="DRAM") as dramp,
            tc.tile_pool(name="persist", bufs=1) as pers,
        ):
            h1tab2 = dramp.tile([NCORES * SHPAD, H], bf)
            cc_in = dramp.tile([4, SHPAD], bf)
            cc_out = dramp.tile([4 * NCORES, SHPAD], bf, addr_space="Shared")
            ccr_in = dramp.tile([H, B_GRAPHS], bf)
            ccr_out = dramp.tile([NCORES * H, B_GRAPHS], bf,
                                 addr_space="Shared")

            h2T = pers.tile([H, SHPAD], bf)
            h1Tl = pers.tile([H, SHPAD], bf)
            w_comb = pers.tile([8, H], bf)
            w_s1 = pers.tile([H, H], bf)
            w_n1 = pers.tile([H, H], bf)
            b1c = pers.tile([H, 1], f32)
            w_i = [pers.tile([H, 4 * H], bf, name=f"w_i{l}") for l in range(2)]
            w_h = [pers.tile([H, 4 * H], bf, name=f"w_h{l}") for l in range(2)]
            b_s4 = [pers.tile([4, H], bf, name=f"b_s4{l}") for l in range(2)]
            sel4 = pers.tile([4, 4 * BL], bf)
            w_o = pers.tile([H, OUT], f32)
            b_o = pers.tile([OUT, 1], f32)
            recrow = pers.tile([128, NG], f32)
            iotab = pers.tile([128, 128], bf)
            identb = pers.tile([128, 128], bf)
            identf = pers.tile([128, 128], f32)
            pooledT = pers.tile([H, B_GRAPHS], f32)

            nc.sync.dma_start(out=w_comb[:], in_=d_wcomb[:])
            nc.sync.dma_start(out=w_s1[:], in_=d_ws1[:])
            nc.sync.dma_start(out=w_n1[:], in_=d_wn1[:])
            nc.sync.dma_start(out=b1c[:], in_=d_b1c[:])
            for l in range(2):
                nc.sync.dma_start(out=w_i[l][:], in_=d_wi[l][:])
                nc.sync.dma_start(out=w_h[l][:], in_=d_wh[l][:])
                nc.sync.dma_start(out=b_s4[l][:], in_=d_bs4[l][:])
            nc.sync.dma_start(out=sel4[:], in_=d_sel4[:])
            nc.sync.dma_start(out=w_o[:], in_=d_wo[:])
            nc.sync.dma_start(out=b_o[:], in_=d_bo[:])
            nc.sync.dma_start(out=recrow[:], in_=d_recrow[:])
            iota_i = pers.tile([128, 128], mybir.dt.int32)
            nc.gpsimd.iota(iota_i[:], pattern=[[1, 128]], base=0,
                           channel_multiplier=0)
            nc.vector.tensor_copy(out=iotab[:], in_=iota_i[:])
            make_identity(nc, identf[:])
            nc.vector.tensor_copy(out=identb[:], in_=identf[:])

            # ---------------- Phase 1: conv1 layered aggregation ----------
            with (
                tc.tile_pool(name="p1l", bufs=2) as lpool,
                tc.tile_pool(name="p1a", bufs=1) as apool,
                tc.tile_pool(name="p1ps", bufs=1, space="PSUM") as pspool1,
            ):
                recq = apool.tile([128, QW], f32)
                nc.sync.dma_start(out=recq[:], in_=d_recq[:])
                acc = pspool1.tile([128, QW], f32, space="PSUM", tag="acc")
                LC = 8                      # layers per DMA chunk
                for k0 in range(0, L, LC):
                    nl = min(LC, L - k0)
                    lt = lpool.tile([128, LC * QW], bf, tag="lay")
                    nc.sync.dma_start(
                        out=lt[:, :nl * QW],
                        in_=d_xlay[:, k0 * QW:(k0 + nl) * QW])
                    for k in range(nl):
                        nc.tensor.matmul(
                            out=acc[:], lhsT=identb[:],
                            rhs=lt[:, k * QW:(k + 1) * QW],
                            start=(k0 + k == 0), stop=(k0 + k == L - 1))
                accb = apool.tile([128, QW], bf)
                nc.vector.tensor_tensor(out=accb[:], in0=acc[:],
                                        in1=recq[:], op=ALU.mult)
                # unpack quad layout [(a f), c] -> agg4T [f, (a c)] via a
                # DRAM bounce (a 2-level partition split is not a valid
                # SBUF access pattern, but is a plain strided DRAM AP)
                qscr = dramp.tile([128, QW], bf)
                nc.sync.dma_start(out=qscr[:], in_=accb[:])
                nc.sync.dma_start(
                    out=cc_in[:].rearrange("f (a c) -> f a c", a=32),
                    in_=qscr[:].rearrange("(a f) c -> f a c", f=4))

            nc.gpsimd.collective_compute(
                "AllGather", mybir.AluOpType.bypass,
                replica_groups=[list(range(NCORES))],
                ins=[cc_in.opt()], outs=[cc_out.opt()],
            )

            # -------- Phase 3: recompute h1 (all ranks), p-major table ----
            with (
                tc.tile_pool(name="p3xa", bufs=2) as xapool,
                tc.tile_pool(name="p3x1", bufs=1) as xapool1,
                tc.tile_pool(name="p3h", bufs=3) as hpool,
                tc.tile_pool(name="p3ps", bufs=3, space="PSUM") as pspool3,
            ):
                # own-shard transposed h1 — needs only local agg4T, so it
                # overlaps the AllGather above
                xal = xapool1.tile([8, SHPAD], bf, tag="xal")
                nc.sync.dma_start(out=xal[0:4, :], in_=d_x4tloc[:])
                nc.sync.dma_start(out=xal[4:8, :], in_=cc_in[:])
                for j0 in range(0, NG, 4):
                    nj = min(4, NG - j0)
                    ps = pspool3.tile([H, 4 * 128], f32, space="PSUM",
                                      tag="psl")
                    for j in range(j0, j0 + nj):
                        nc.tensor.matmul(
                            out=ps[:, (j - j0) * 128:(j - j0 + 1) * 128],
                            lhsT=w_comb[:],
                            rhs=xal[:, j * 128:(j + 1) * 128],
                            start=True, stop=True)
                    nc.scalar.activation(
                        h1Tl[:, j0 * 128:(j0 + nj) * 128],
                        ps[:, :nj * 128], AF.Relu)

                for r in range(NCORES):
                    xa = xapool.tile([8, SHPAD], bf, tag="xa")
                    nc.sync.dma_start(out=xa[0:4, :],
                                      in_=d_x4T[:, r * SHPAD:(r + 1) * SHPAD])
                    nc.sync.dma_start(out=xa[4:8, :],
                                      in_=cc_out[4 * r:4 * r + 4, :])
                    for half in range(2):
                        h0 = half * 49
                        hb = hpool.tile([128, 49 * 128], bf, tag="hrow")
                        for bi, j0 in enumerate(range(h0, h0 + 49, 4)):
                            nj = min(4, h0 + 49 - j0)
                            ps = pspool3.tile([128, 4 * H], f32, space="PSUM",
                                              tag="psr")
                            for j in range(j0, j0 + nj):
                                nc.tensor.matmul(
                                    out=ps[:, (j - j0) * H:(j - j0 + 1) * H],
                                    lhsT=xa[:, j * 128:(j + 1) * 128],
                                    rhs=w_comb[:], start=True, stop=True)
                            dst = hb[:, (j0 - h0) * 128:(j0 - h0 + nj) * 128]
                            if bi % 2 == 0:
                                nc.scalar.activation(dst, ps[:, :nj * H],
                                                     AF.Relu)
                            else:
                                nc.vector.tensor_scalar(
                                    out=dst, in0=ps[:, :nj * H], scalar1=0.0,
                                    scalar2=None, op0=ALU.max)
                        # p-major write: rows p*98+h0+(0..48), 12.5KB/desc
                        nc.sync.dma_start(
                            out=h1tab2[r * SHPAD:(r + 1) * SHPAD, :]
                                .rearrange("(p j) h -> p (j h)", p=128)
                                [:, h0 * 128:(h0 + 49) * 128],
                            in_=hb[:, :])

            # ---------------- Phase 4: conv2 -----------------------------
            with (
                tc.tile_pool(name="p4g", bufs=3) as gpool,
                tc.tile_pool(name="p4oh", bufs=2) as ohpool,
                tc.tile_pool(name="p4m", bufs=3) as mpool,
                tc.tile_pool(name="p4ps", bufs=2, space="PSUM") as pspool4a,
                tc.tile_pool(name="p4ps2", bufs=2, space="PSUM") as pspool4b,
                tc.tile_pool(name="p4t", bufs=3) as tpool,
            ):
                KRMAX = max(kr for bruns in gmeta for _, _, kr in bruns)
                agg2acc = pers.tile([128, NG * 128], bf)
                # b-major: block b's gathers depend only on h1 of cores
                # 2b/2b+1, overlapping conv2 with the tail of phase 3
                for b in range(NB):
                    for bruns, sup in zip(gmeta, sups):
                        _, run_base, kr = bruns[b]
                        n_idx = kr * 128
                        it = mpool.tile([128, (KRMAX * 128) // 16], i16,
                                        tag="idx")
                        nc.sync.dma_start(
                            out=it[:, :n_idx // 16],
                            in_=d_idx[:, run_base * 8:
                                      run_base * 8 + n_idx // 16])
                        dm_t = mpool.tile([128, KRMAX], bf, tag="dm")
                        nc.sync.dma_start(
                            out=dm_t[:, :kr],
                            in_=d_dmv[:, run_base:run_base + kr])
                        gt = gpool.tile([128, KRMAX * 128], bf, tag="g")
                        for o in range(0, n_idx, GCALL):
                            nn_ = min(GCALL, n_idx - o)
                            nc.gpsimd.dma_gather(
                                out_ap=gt[:, o:o + nn_]
                                    .rearrange("p (k h) -> p k h", h=H),
                                in_ap=h1tab2[b * BLK2:(b + 1) * BLK2, :],
                                idxs_ap=it[:, o // 16:(o + nn_) // 16],
                                num_idxs=nn_,
                                num_idxs_reg=nn_,
                                elem_size=H,
                            )
                        oh_t = ohpool.tile([128, KRMAX * 128], bf, tag="oh")
                        for bi, k0 in enumerate(range(0, kr, 16)):
                            nb_ = min(16, kr - k0)
                            eng = nc.vector
                            eng.tensor_tensor(
                                out=oh_t[:, k0 * 128:(k0 + nb_) * 128],
                                in0=dm_t[:, k0:k0 + nb_]
                                    .to_broadcast([128, nb_, 128]),
                                in1=iotab[:].rearrange("p (k s) -> p k s", k=1)
                                    .to_broadcast([128, nb_, 128]),
                                op=ALU.is_equal,
                            )
                        ps = pspool4a.tile([128, GS * 128], f32, space="PSUM",
                                           tag="aggps", name="aggps")
                        ng_ = len(sup)
                        c0 = sup[0] * 128
                        if b > 0:
                            for z0 in range(0, ng_ * 128, 512):
                                zw = min(512, ng_ * 128 - z0)
                                nc.tensor.matmul(
                                    out=ps[:, z0:z0 + zw], lhsT=identb[:],
                                    rhs=agg2acc[:, c0 + z0:c0 + z0 + zw],
                                    start=True, stop=False)
                        for gi, g in enumerate(sup):
                            for kk in range(Kf[g, b]):
                                chunk = int(chunk_base[g, b]) + kk
                                off = (chunk - run_base) * 128
                                nc.tensor.matmul(
                                    out=ps[:, gi * 128:(gi + 1) * 128],
                                    lhsT=oh_t[:, off:off + 128],
                                    rhs=gt[:, off:off + H],
                                    start=(b == 0 and kk == 0),
                                    stop=(b == NB - 1 and kk == Kf[g, b] - 1))
                        if b != NB - 1:
                            nc.scalar.activation(
                                agg2acc[:, c0:c0 + ng_ * 128],
                                ps[:, :ng_ * 128], AF.Copy)
                            continue
                        for gi, g in enumerate(sup):
                            w = 128 if g < NG - 1 else SH - 128 * (NG - 1)
                            aggn = tpool.tile([128, H], bf, tag="aggn")
                            nc.scalar.activation(
                                aggn[:w, :], ps[:w, gi * 128:(gi + 1) * 128],
                                AF.Copy, scale=recrow[:w, g:g + 1])
                            pst = pspool4b.tile([128, 128], bf, space="PSUM",
                                                tag="tr")
                            nc.tensor.transpose(out=pst[:, :w],
                                                in_=aggn[:w, :],
                                                identity=identb[:w, :w])
                            aggnTc = tpool.tile([H, 128], bf, tag="aggnTc")
                            nc.scalar.activation(aggnTc[:, :w], pst[:, :w],
                                                 AF.Copy)
                            ps2 = pspool4b.tile([H, 128], f32, space="PSUM",
                                                tag="h2")
                            nc.tensor.matmul(out=ps2[:, :w], lhsT=w_s1[:],
                                             rhs=h1Tl[:, g * 128:g * 128 + w],
                                             start=True, stop=False)
                            nc.tensor.matmul(out=ps2[:, :w], lhsT=w_n1[:],
                                             rhs=aggnTc[:, :w], start=False,
                                             stop=True)
                            nc.scalar.activation(h2T[:, g * 128:g * 128 + w],
                                                 ps2[:, :w], AF.Relu,
                                                 bias=b1c[:, 0:1])

            # ---------------- Phase 5: LSTM ------------------------------
            with (
                tc.tile_pool(name="p5s", bufs=6) as spool,
                tc.tile_pool(name="p5ps", bufs=4, space="PSUM") as pspool5,
            ):
                hprev = [None, None]
                cprev = [None, None]

                def lstm_cell(t, l, xin):
                    ps = pspool5.tile([H, 4 * BL], f32, space="PSUM",
                                      tag=f"g{l}")
                    nc.tensor.matmul(out=ps[:], lhsT=b_s4[l][:],
                                     rhs=sel4[:], start=True, stop=False)
                    for q in range(4):
                        nc.tensor.matmul(
                            out=ps[:, q * BL:(q + 1) * BL],
                            lhsT=w_i[l][:, q * H:(q + 1) * H],
                            rhs=xin, start=False, stop=(t == 0))
                    if t > 0:
                        for q in range(4):
                            nc.tensor.matmul(
                                out=ps[:, q * BL:(q + 1) * BL],
                                lhsT=w_h[l][:, q * H:(q + 1) * H],
                                rhs=hprev[l][:], start=False, stop=True)
                    # gate cols: g [0:125], i [125:250], f [250:375],
                    # o [375:500]
                    tg = spool.tile([H, BL], f32, tag=f"tg{l}")
                    nc.scalar.activation(tg[:], ps[:, 0:BL], AF.Tanh)
                    sig = spool.tile([H, 2 * BL], f32, tag=f"sig{l}")
                    nc.scalar.activation(sig[:], ps[:, BL:3 * BL], AF.Sigmoid)
                    cnew = spool.tile([H, BL], f32, tag=f"c{l}")
                    if t > 0:
                        nc.vector.tensor_tensor(out=cnew[:],
                                                in0=sig[:, BL:2 * BL],
                                                in1=cprev[l][:], op=ALU.mult)
                        t1 = spool.tile([H, BL], f32, tag=f"t1{l}")
                        nc.vector.tensor_tensor(out=t1[:], in0=sig[:, 0:BL],
                                                in1=tg[:], op=ALU.mult)
                        nc.vector.tensor_tensor(out=cnew[:], in0=cnew[:],
                                                in1=t1[:], op=ALU.add)
                    else:
                        nc.vector.tensor_tensor(out=cnew[:], in0=sig[:, 0:BL],
                                                in1=tg[:], op=ALU.mult)
                    sgo = spool.tile([H, BL], f32, tag=f"so{l}")
                    nc.scalar.activation(sgo[:], ps[:, 3 * BL:4 * BL],
                                         AF.Sigmoid)
                    tc_ = spool.tile([H, BL], f32, tag=f"tc{l}")
                    nc.scalar.activation(tc_[:], cnew[:], AF.Tanh)
                    hnew = spool.tile([H, BL], bf, tag=f"h{l}")
                    nc.vector.tensor_tensor(out=hnew[:], in0=sgo[:],
                                            in1=tc_[:], op=ALU.mult)
                    cprev[l] = cnew
                    hprev[l] = hnew
                    return hnew

                # software pipeline: layer 0 runs two steps ahead of
                # layer 1 so its activations fill layer-0's recurrence gaps
                import concourse.mybir as _mb
                h0q = [lstm_cell(0, 0, h2T[:, 0:BL]),
                       lstm_cell(1, 0, h2T[:, BL:2 * BL])]
                for t in range(T):
                    if t + 2 < T:
                        h0q.append(lstm_cell(t + 2, 0,
                                   h2T[:, (t + 2) * BL:(t + 3) * BL]))
                    h1c = lstm_cell(t, 1, h0q[t][:])
                    nc.vector.tensor_reduce(
                        out=pooledT[:, t:t + 1], in_=h1c[:],
                        axis=_mb.AxisListType.X, op=ALU.add)

            # ---------------- Phase 6: head ------------------------------
            with (
                tc.tile_pool(name="p6", bufs=1) as hp,
                tc.tile_pool(name="p6ps", bufs=1, space="PSUM") as psp,
            ):
                pooledb = hp.tile([H, B_GRAPHS], bf)
                nc.vector.tensor_copy(out=pooledb[:], in_=pooledT[:])
                nc.sync.dma_start(out=ccr_in[:], in_=pooledb[:])
                nc.gpsimd.collective_compute(
                    "AllGather", mybir.AluOpType.bypass,
                    replica_groups=[list(range(NCORES))],
                    ins=[ccr_in.opt()], outs=[ccr_out.opt()],
                )
                prT8 = hp.tile([H, NCORES * B_GRAPHS], bf)
                nc.sync.dma_start(
                    out=prT8[:].rearrange("h (r b) -> h r b", r=NCORES),
                    in_=ccr_out[:].rearrange("(r h) b -> h r b", h=H))
                prT = hp.tile([H, B_GRAPHS], f32)
                nc.vector.tensor_tensor(
                    out=prT[:], in0=prT8[:, 0:B_GRAPHS],
                    in1=prT8[:, B_GRAPHS:2 * B_GRAPHS], op=ALU.add)
                for r in range(2, NCORES):
                    nc.vector.tensor_tensor(
                        out=prT[:], in0=prT[:],
                        in1=prT8[:, r * B_GRAPHS:(r + 1) * B_GRAPHS],
                        op=ALU.add)
                psl = psp.tile([OUT, B_GRAPHS], f32, space="PSUM", tag="lg")
                nc.tensor.matmul(out=psl[:], lhsT=w_o[:], rhs=prT[:],
                                 start=True, stop=True)
                lg = hp.tile([OUT, B_GRAPHS], f32)
                nc.vector.tensor_scalar(out=lg[:], in0=psl[:],
                                        scalar1=b_o[:, 0:1], scalar2=None,
                                        op0=ALU.add)
                pst = psp.tile([B_GRAPHS, OUT], f32, space="PSUM", tag="lgt")
                nc.tensor.transpose(out=pst[:], in_=lg[:],
                                    identity=identf[0:OUT, 0:OUT])
                z = hp.tile([B_GRAPHS, OUT], f32)
                nc.vector.tensor_copy(out=z[:], in_=pst[:])
                m = hp.tile([B_GRAPHS, 1], f32)
                nc.vector.tensor_reduce(out=m[:], in_=z[:],
                                        axis=mybir.AxisListType.X, op=ALU.max)
                negm = hp.tile([B_GRAPHS, 1], f32)
                nc.vector.tensor_scalar(out=negm[:], in0=m[:], scalar1=-1.0,
                                        scalar2=None, op0=ALU.mult)
                e = hp.tile([B_GRAPHS, OUT], f32)
                se = hp.tile([B_GRAPHS, 1], f32)
                nc.scalar.activation(e[:], z[:], AF.Exp, bias=negm[:, 0:1],
                                     accum_out=se[:])
                ls = hp.tile([B_GRAPHS, 1], f32)
                nc.scalar.activation(ls[:], se[:], AF.Ln)
                o_sb = hp.tile([B_GRAPHS, OUT], f32)
                nc.vector.tensor_scalar(out=o_sb[:], in0=z[:],
                                        scalar1=m[:, 0:1], scalar2=ls[:, 0:1],
                                        op0=ALU.subtract, op1=ALU.subtract)
                nc.sync.dma_start(out=d_out[:], in_=o_sb[:])

    nc.compile()
    return nc


# --------------------------------------------------------------------------
# PJRT runner (built once, reused across calls)
# --------------------------------------------------------------------------
class _Runner:
    def __init__(self, nc, n_cores):
        import jax
        import concourse.mybir as mybir
        from jax.sharding import Mesh, PartitionSpec
        from jax.experimental.shard_map import shard_map
        from concourse.bass2jax import (
            _bass_exec_p, install_neuronx_cc_hook, partition_id_tensor)

        install_neuronx_cc_hook()
        self.n_cores = n_cores
        in_names, out_names, out_avals, zero_outs = [], [], [], []
        pname = nc.partition_id_tensor.name if nc.partition_id_tensor else None
        for alloc in nc.m.functions[0].allocations:
            if not isinstance(alloc, mybir.MemoryLocationSet):
                continue
            name = alloc.memorylocations[0].name
            if alloc.kind == "ExternalInput":
                if name != pname:
                    in_names.append(name)
            elif alloc.kind == "ExternalOutput":
                shape = tuple(alloc.tensor_shape)
                dtype = mybir.dt.np(alloc.dtype)
                out_names.append(name)
                out_avals.append(jax.core.ShapedArray(shape, dtype))
                zero_outs.append(np.zeros(shape, dtype))
        self.in_names, self.out_names = in_names, out_names
        self.out_avals, self.zero_outs = out_avals, zero_outs
        n_params, n_outs = len(in_names), len(out_names)
        all_in = list(in_names) + list(out_names) + ([pname] if pname else [])

        def _body(*args):
            operands = list(args)
            if pname is not None:
                operands.append(partition_id_tensor())
            return tuple(_bass_exec_p.bind(
                *operands, out_avals=tuple(out_avals),
                in_names=tuple(all_in), out_names=tuple(out_names),
                lowering_input_output_aliases=(),
                sim_require_finite=True, sim_require_nnan=True, nc=nc))

        devices = jax.devices()[:n_cores]
        mesh = Mesh(np.asarray(devices), ("core",))
        self._jax = jax
        self.sharded = jax.jit(
            shard_map(_body, mesh=mesh,
                      in_specs=(PartitionSpec("core"),) * (n_params + n_outs),
                      out_specs=(PartitionSpec("core"),) * n_outs,
                      check_rep=False),
            donate_argnums=tuple(range(n_params, n_params + n_outs)),
            keep_unused=True)

    def concat_inputs(self, in_maps):
        return [np.concatenate([np.asarray(m[nm]) for m in in_maps], axis=0)
                for nm in self.in_names]

    def run(self, concat_in):
        zeros = [np.zeros((self.n_cores * z.shape[0], *z.shape[1:]), z.dtype)
                 for z in self.zero_outs]
        out = self.sharded(*concat_in, *zeros)
        self._jax.block_until_ready(out)
        return out

    def split(self, out_arrs):
        return [{nm: np.asarray(out_arrs[i]).reshape(
            self.n_cores, *self.out_avals[i].shape)[c]
            for i, nm in enumerate(self.out_names)}
            for c in range(self.n_cores)]


def kernel(**inputs):
    shared, percore, meta, K, gmeta, NCH, chunk_base, L = _host_prep(inputs)
    if meta not in _BUILT:
        nc = _build_nc(K, gmeta, NCH, chunk_base, L)
        _BUILT[meta] = (nc, _Runner(nc, NCORES))
    nc, runner = _BUILT[meta]
    in_maps = [dict(shared, **percore[c]) for c in range(NCORES)]
    ci = runner.concat_inputs(in_maps)
    outs = runner.split(runner.run(ci))
    return np.asarray(outs[0]["out"], np.float32)
